# revision 21
# baseline (speedup 1.0000x reference)
"""DiffPoolNet on 8 TRN2 NeuronCores (Bass/Tile).

Sharding: data-parallel over graphs, 32 graphs per core. Per graph the dense
256x256 adjacency (transposed, degree-normalized) lives in SBUF as fp16,
built on-device by the GPSIMD local_scatter instruction from host-bucketed
edge index lists -- the dense adjacency never touches HBM.

Activations are feature-major ([channels, nodes]); the pool/embed GNN blocks
are stacked on the partition axis (pool = 0:64, embed = 64:128) so one matmul
with block-diagonal weights serves both blocks.

Training-mode BatchNorm needs global stats: each of the 9 BN layers does a
[<=128,2] AllReduce of (sum, sumsq) across the 8 cores. The BN affine is
folded into the *next* layer's weights/bias (gamma-scaled weights, bias
absorbing the beta terms), so the heavy per-graph adjacency matmuls of layer
l+1 depend only on pre-BN activations and can overlap the collective.

All matmul operands are fp16 (fp32 PSUM accumulate); measured end-to-end
error vs the fp32 reference is ~8e-4 max-rel.
"""
import numpy as np

import concourse.bacc as bacc
import concourse.mybir as mybir
import concourse.tile as tile
from concourse import library_config
from concourse.bass_utils import run_bass_kernel_spmd

F32 = mybir.dt.float32
F16 = mybir.dt.float16
I16 = mybir.dt.int16
AF = mybir.ActivationFunctionType
OP = mybir.AluOpType
AX = mybir.AxisListType

import os
N_CORES = int(os.environ.get("KNC", "8"))
B = 256
NG = int(os.environ.get("KNG", str(B // N_CORES)))    # graphs per core
N1, F0, H = 256, 128, 64
ST = 2 * H
K1, K2, N2, N3 = 64, 16, 64, 16
NCLS = 10
EPS = 1e-5
NT1, NT2, NT3 = B * N1, B * N2, B * N3

_CACHE = {}


# --------------------------------------------------------------------------
# Host prep
# --------------------------------------------------------------------------

def _prep(x, batch, edge_index, params):
    x = np.asarray(x)
    batch = np.asarray(batch)
    e0, e1 = np.asarray(edge_index)

    counts = np.bincount(batch, minlength=B)
    assert counts.shape[0] == B and np.all(counts == N1), "expects 256 nodes/graph"
    starts = np.concatenate([[0], np.cumsum(counts)[:-1]]).astype(np.int64)
    pos = np.arange(batch.shape[0], dtype=np.int64) - starts[batch]

    g = batch[e0].astype(np.int64)
    u = pos[e0]
    v = pos[e1]
    key = np.unique((g << 32) | (u << 16) | v)
    g = (key >> 32)
    u = (key >> 16) & 0xFFFF
    v = key & 0xFFFF

    deg = np.bincount(g * N1 + u, minlength=B * N1)
    assert deg.min() >= 1, "zero out-degree node: rank-1 delta path not built"

    p = v & 127
    elem = ((v >> 7) << 8) + u
    bucket = (g * 128 + p).astype(np.int64)
    order = np.argsort(bucket, kind="stable")
    bucket_s = bucket[order]
    elem_s = elem[order]
    bc = np.bincount(bucket_s, minlength=B * 128)
    bstart = np.concatenate([[0], np.cumsum(bc)])
    rank = np.arange(elem_s.shape[0]) - bstart[bucket_s]
    nidx = max(64, (int(bc.max()) + 1) & ~1)
    assert nidx <= 512

    idx_all = np.full((B * 128, nidx), -1, np.int16)
    idx_all[bucket_s, rank] = elem_s.astype(np.int16)
    idx_all = idx_all.reshape(B, 128, nidx)

    P = params
    W = {}

    def cat_T(a, b):
        return np.concatenate([np.asarray(a).T, np.asarray(b).T], axis=1).astype(np.float32)

    def blk_T(a, b):
        a, b = np.asarray(a), np.asarray(b)
        fia, foa = a.shape[1], a.shape[0]
        fib, fob = b.shape[1], b.shape[0]
        w = np.zeros((fia + fib, foa + fob), np.float32)
        w[:fia, :foa] = a.T
        w[fia:, foa:] = b.T
        return w

    def cat_v(a, b):
        return np.concatenate([np.asarray(a), np.asarray(b)]).astype(np.float32)[:, None]

    def col(a):
        return np.asarray(a).astype(np.float32)[:, None]

    for ph, pp_, pe_ in (("1", "gnn1_pool", "gnn1_embed"),
                         ("2", "gnn2_pool", "gnn2_embed")):
        cp, ce = P[pp_], P[pe_]
        c1r = cat_T(cp["conv1"]["Wr"], ce["conv1"]["Wr"])        # [fi, 128]
        c1o = cat_T(cp["conv1"]["Wroot"], ce["conv1"]["Wroot"])
        if ph == "2":   # fi = 192 > 128: split partition chunks
            W["w2c1r_a"], W["w2c1r_b"] = c1r[:128], c1r[128:]
            W["w2c1root_a"], W["w2c1root_b"] = c1o[:128], c1o[128:]
        else:
            W["w1c1r"], W["w1c1root"] = c1r, c1o
        W[f"b{ph}c1"] = cat_v(cp["conv1"]["br"], ce["conv1"]["br"])
        for l in (2, 3):
            W[f"w{ph}c{l}r"] = blk_T(cp[f"conv{l}"]["Wr"], ce[f"conv{l}"]["Wr"])
            W[f"w{ph}c{l}root"] = blk_T(cp[f"conv{l}"]["Wroot"], ce[f"conv{l}"]["Wroot"])
            W[f"b{ph}c{l}"] = cat_v(cp[f"conv{l}"]["br"], ce[f"conv{l}"]["br"])
        for l in (1, 2, 3):
            W[f"bn{ph}g{l}"] = cat_v(cp[f"bn{l}"]["g"], ce[f"bn{l}"]["g"])
            W[f"bn{ph}b{l}"] = cat_v(cp[f"bn{l}"]["b"], ce[f"bn{l}"]["b"])
        lw = np.asarray(cp["lin"]["W"]).T.astype(np.float32)
        pw = 64 if ph == "1" else 16    # pool conv3 output width
        csz = [64, 64, pw]
        off = 0
        for i in range(3):
            W[f"wlin{ph}_{i}"] = lw[off:off + csz[i]]
            off += csz[i]
        W[f"blin{ph}"] = col(cp["lin"]["b"])

    c3 = P["gnn3_embed"]
    w = np.asarray(c3["conv1"]["Wr"]).T.astype(np.float32)
    W["w3c1r_a"], W["w3c1r_b"] = w[:128], w[128:]
    w = np.asarray(c3["conv1"]["Wroot"]).T.astype(np.float32)
    W["w3c1root_a"], W["w3c1root_b"] = w[:128], w[128:]
    W["b3c1"] = col(c3["conv1"]["br"])
    for l in (2, 3):
        W[f"w3c{l}r"] = np.asarray(c3[f"conv{l}"]["Wr"]).T.astype(np.float32)
        W[f"w3c{l}root"] = np.asarray(c3[f"conv{l}"]["Wroot"]).T.astype(np.float32)
        W[f"b3c{l}"] = col(c3[f"conv{l}"]["br"])
    for l in (1, 2, 3):
        W[f"bn3g{l}"] = col(c3[f"bn{l}"]["g"])
        W[f"bn3b{l}"] = col(c3[f"bn{l}"]["b"])
    W["ident16"] = np.eye(128, dtype=np.float32)
    W["ident32"] = np.eye(128, dtype=np.float32)
    w = np.asarray(P["lin1"]["W"]).T.astype(np.float32)
    W["wlin1f_a"], W["wlin1f_b"] = w[:128], w[128:]
    W["blin1f"] = col(P["lin1"]["b"])
    W["wlin2f"] = np.asarray(P["lin2"]["W"]).T.astype(np.float32)
    W["blin2f"] = col(P["lin2"]["b"])

    in_maps = []
    for c in range(N_CORES):
        xs = x[c * NG * N1:(c + 1) * NG * N1].astype(np.float32)
        m = {
            "x_nm": np.ascontiguousarray(xs),
            "x_fm": np.ascontiguousarray(xs.T),
            "scat": np.ascontiguousarray(
                idx_all[c * NG:(c + 1) * NG].transpose(1, 0, 2).reshape(128, NG * nidx)),
        }
        m.update(W)
        in_maps.append(m)
    return in_maps, nidx, {k: tuple(w.shape) for k, w in W.items()}


# --------------------------------------------------------------------------
# Device program
# --------------------------------------------------------------------------

def _build(nidx, wshapes):
    nc = bacc.Bacc("TRN2", target_bir_lowering=False, debug=False,
                   num_devices=N_CORES)
    x_nm = nc.dram_tensor("x_nm", [NG * N1, F0], F32, kind="ExternalInput")
    x_fm = nc.dram_tensor("x_fm", [F0, NG * N1], F32, kind="ExternalInput")
    scat = nc.dram_tensor("scat", [128, NG * nidx], I16, kind="ExternalInput")
    wd = {k: nc.dram_tensor(k, list(s), F32, kind="ExternalInput")
          for k, s in wshapes.items()}
    out_d = nc.dram_tensor("out", [NG, NCLS], F32, kind="ExternalOutput")

    RG = [list(range(N_CORES))]

    with tile.TileContext(nc) as tc:
        with (
            tc.tile_pool(name="persist", bufs=1) as pp,
            tc.tile_pool(name="wpool", bufs=1) as wp,
            tc.tile_pool(name="wsp", bufs=1) as wsp,
            tc.tile_pool(name="stream", bufs=2) as stp,
            tc.tile_pool(name="sb", bufs=2) as sb,
            tc.tile_pool(name="sb2", bufs=2) as sb2,
            tc.tile_pool(name="sbr", bufs=1) as sbr,
            tc.tile_pool(name="sbsm", bufs=3) as sm,
            tc.tile_pool(name="stats", bufs=1) as sp,
            tc.tile_pool(name="psA", bufs=3, space="PSUM") as psA,
            tc.tile_pool(name="psT", bufs=3, space="PSUM") as psT,
            tc.tile_pool(name="psS", bufs=2, space="PSUM") as psS,
            tc.tile_pool(name="dram", bufs=1, space="DRAM") as dr,
        ):
            nc.gpsimd.load_library(library_config.local_scatter)

            ones_col = wp.tile([128, 1], F16, tag="ones_col", name="ones_col")
            nc.vector.memset(ones_col[:], 1.0)
            ones_row = wp.tile([1, 128], F16, tag="ones_row", name="ones_row")
            nc.vector.memset(ones_row[:], 1.0)
            ones_dat = wp.tile([128, nidx], F16, tag="ones_dat", name="ones_dat")
            nc.vector.memset(ones_dat[:], 1.0)

            wt = {}
            for k, s in wshapes.items():
                if k[0] == "b" or k == "ident32":   # f32 tiles
                    t = wp.tile(list(s), F32, tag=k, name=k)
                    nc.sync.dma_start(t[:], wd[k][:, :])
                else:
                    t = wp.tile(list(s), F16, tag=k, name=k)
                    nc.gpsimd.dma_start(t[:], wd[k][:, :])
                wt[k] = t
            ident16 = wt["ident16"]
            ident32 = wt["ident32"]

            # ---------------- persistent arrays ----------------
            # pp: survives the phase1->phase2 boundary
            h1a = pp.tile([128, NG * N2], F16, tag="h1a", name="h1a")
            h1b = pp.tile([64, NG * N2], F16, tag="h1b", name="h1b")
            A2 = pp.tile([N2, NG * N2], F16, tag="A2", name="A2")
            t2col = pp.tile([N2, NG], F32, tag="t2col", name="t2col")
            h1n = pp.tile([N2, NG, 3 * H], F16, tag="h1n", name="h1n")
            ST2L = [128, 128, 80]   # phase-2 stacked widths (pool conv3 = 16ch)
            PW2 = [64, 64, 16]      # phase-2 pool widths

            # p1: phase-1 only, released before the phase-2/3 arrays are live
            p1 = tc.alloc_tile_pool(name="p1", bufs=1)
            A1 = p1.tile([128, NG, 2, N1], F16, tag="A1", name="A1")
            t1col = p1.tile([128, NG, 2], F32, tag="t1col", name="t1col")
            r1 = [p1.tile([ST, NG * N1], F16, tag=f"r1_{l}", name=f"r1_{l}") for l in range(3)]
            xr1 = [p1.tile([128, NG, 2, ST], F16, tag=f"xr1_{l}", name=f"xr1_{l}") for l in range(3)]
            s1 = p1.tile([128, NG, 2, K1], F16, tag="s1", name="s1")
            cs1 = p1.tile([1, NG * K1], F16, tag="cs1", name="cs1")

            NPR = NG // 2                 # phase-1 pair count
            G2W = min(8, NG)              # phase-2 graphs per group
            NGR2 = NG // G2W
            sum1 = [sp.tile([ST, NPR], F32, tag=f"sum1_{l}", name=f"sum1_{l}") for l in range(3)]
            sq1 = [sp.tile([ST, NPR], F32, tag=f"sq1_{l}", name=f"sq1_{l}") for l in range(3)]
            sum2 = [sp.tile([ST2L[l], NGR2], F32, tag=f"sum2_{l}", name=f"sum2_{l}") for l in range(3)]
            sq2 = [sp.tile([ST2L[l], NGR2], F32, tag=f"sq2_{l}", name=f"sq2_{l}") for l in range(3)]
            sum3 = [sp.tile([H, 1], F32, tag=f"sum3_{l}", name=f"sum3_{l}") for l in range(3)]
            sq3 = [sp.tile([H, 1], F32, tag=f"sq3_{l}", name=f"sq3_{l}") for l in range(3)]

            x_nm_r = x_nm.ap().rearrange("(g h p) f -> g p h f", g=NG, h=2, p=128)

            # ---------------- adjacency build ----------------
            for g in range(NG):
                sct = stp.tile([128, nidx], I16, tag="sct", name="sct")
                nc.sync.dma_start(sct[:], scat[:, g * nidx:(g + 1) * nidx])
                araw = sb2.tile([128, 2, N1], F16, tag="araw", name="araw")
                nc.gpsimd.local_scatter(
                    out_ap=araw[:], data_ap=ones_dat[:], idxs_ap=sct[:],
                    channels=128, num_elems=2 * N1, num_idxs=nidx)
                dps = psS.tile([1, N1], F32, space="PSUM", tag="S", name="S")
                for vh in range(2):
                    nc.tensor.matmul(dps[:], lhsT=ones_col[:], rhs=araw[:, vh, :],
                                     start=(vh == 0), stop=(vh == 1))
                trow = sm.tile([1, N1], F16, tag="trow", name="trow")
                nc.vector.tensor_scalar_max(trow[:], dps[:], 1.0)
                dcol = psS.tile([128, 2], F32, space="PSUM", tag="S", name="S")
                for uh in range(2):
                    for vh in range(2):
                        nc.tensor.matmul(
                            dcol[:, uh:uh + 1],
                            lhsT=araw[:, vh, uh * 128:(uh + 1) * 128],
                            rhs=ones_col[:], start=(vh == 0), stop=(vh == 1))
                nc.vector.tensor_scalar_max(t1col[:, g, :], dcol[:], 1.0)
                bps = psA.tile([128, 2, N1], F32, space="PSUM", tag="A", name="A")
                for vh in range(2):
                    nc.tensor.matmul(bps[:, vh, :], lhsT=ones_row[:], rhs=trow[:],
                                     start=True, stop=True)
                rdb = sbr.tile([128, 2, N1], F32, tag="rdb", name="rdb")
                nc.vector.reciprocal(rdb[:], bps[:])
                nc.vector.tensor_tensor(out=A1[:, g, :, :], in0=araw[:], in1=rdb[:],
                                        op=OP.mult)

            # ---------------- helpers ----------------
            def bn_sync(tag, C, sumc, sqc, NT, gk, bk):
                stg = sm.tile([C, 2], F32, tag="bn_stg", name="bn_stg")
                if sumc.shape[1] > 1:
                    nc.vector.tensor_reduce(stg[:, 0:1], sumc[:], AX.X, OP.add)
                    nc.vector.tensor_reduce(stg[:, 1:2], sqc[:], AX.X, OP.add)
                else:
                    nc.vector.tensor_copy(stg[:, 0:1], sumc[:])
                    nc.vector.tensor_copy(stg[:, 1:2], sqc[:])
                bin_ = dr.tile([C, 2], F32, tag=f"cc_in_{tag}", name=f"cc_in_{tag}")
                bout = dr.tile([C, 2], F32, tag=f"cc_out_{tag}", name=f"cc_out_{tag}")
                nc.gpsimd.dma_start(bin_[:], stg[:])
                nc.gpsimd.collective_compute(
                    "AllReduce", OP.add, replica_groups=RG,
                    ins=[bin_.opt()], outs=[bout.opt()])
                red = sm.tile([C, 2], F32, tag="bn_red", name="bn_red")
                nc.gpsimd.dma_start(red[:], bout[:])
                mu = sm.tile([C, 1], F32, tag="bn_mu", name="bn_mu")
                nc.vector.tensor_scalar_mul(mu[:], red[:, 0:1], 1.0 / NT)
                var = sm.tile([C, 1], F32, tag="bn_var", name="bn_var")
                nc.vector.tensor_tensor(out=var[:], in0=mu[:], in1=mu[:], op=OP.mult)
                msq = sm.tile([C, 1], F32, tag="bn_msq", name="bn_msq")
                nc.vector.tensor_scalar(out=msq[:], in0=red[:, 1:2], scalar1=1.0 / NT,
                                        scalar2=EPS, op0=OP.mult, op1=OP.add)
                nc.vector.tensor_tensor(out=var[:], in0=msq[:], in1=var[:], op=OP.subtract)
                rec = sm.tile([C, 1], F32, tag="bn_rec", name="bn_rec")
                nc.vector.reciprocal(rec[:], var[:])
                rstd = sm.tile([C, 1], F32, tag="bn_rstd", name="bn_rstd")
                nc.scalar.activation(rstd[:], rec[:], AF.Sqrt)
                gam = sm.tile([C, 1], F32, tag="bn_gam", name="bn_gam")
                nc.vector.tensor_tensor(out=gam[:], in0=wt[gk][:], in1=rstd[:], op=OP.mult)
                bet = sm.tile([C, 1], F32, tag="bn_bet", name="bn_bet")
                nc.vector.tensor_tensor(out=bet[:], in0=mu[:], in1=gam[:], op=OP.mult)
                nc.vector.tensor_tensor(out=bet[:], in0=wt[bk][:], in1=bet[:], op=OP.subtract)
                bet16 = sm.tile([C, 1], F16, tag="bn_bet16", name="bn_bet16")
                nc.vector.tensor_copy(bet16[:], bet[:])
                brps = psS.tile([1, C], F16, space="PSUM", tag="S", name="S")
                nc.tensor.transpose(brps[:], bet16[:], ident16[:C, :C])
                brow = sm.tile([1, C], F16, tag="bn_brow", name="bn_brow")
                nc.vector.tensor_copy(brow[:], brps[:])
                return gam, bet16, brow

            def scale_w(wk, gam):
                w = wt[wk]
                K = w.shape[0]
                ws = wsp.tile(list(w.shape), F16, tag=f"ws_{wk}", name=f"ws_{wk}")
                nc.vector.tensor_tensor(out=ws[:], in0=w[:],
                                        in1=gam[:K, 0:1].to_broadcast(tuple(w.shape)),
                                        op=OP.mult)
                return ws

            def fold_bias(wrk, wrootk, bk, bet16):
                K = wt[wrk].shape[0]
                C = wt[wrk].shape[1]
                ps = psS.tile([C, 1], F32, space="PSUM", tag="S", name="S")
                nc.tensor.matmul(ps[:], lhsT=wt[wrk][:], rhs=bet16[:K], start=True, stop=False)
                nc.tensor.matmul(ps[:], lhsT=wt[wrootk][:], rhs=bet16[:K], start=False, stop=True)
                bias = sm.tile([C, 1], F32, tag="fb_bias", name="fb_bias")
                nc.vector.tensor_tensor(out=bias[:], in0=ps[:], in1=wt[bk][:], op=OP.add)
                return bias

            # ======================== PHASE 1 ========================
            gam1, bet1, brow1 = [None] * 3, [None] * 3, [None] * 3
            for l in range(3):
                if l == 0:
                    wr, wroot, bias = wt["w1c1r"], wt["w1c1root"], wt["b1c1"]
                else:
                    wr = scale_w(f"w1c{l + 1}r", gam1[l - 1])
                    wroot = scale_w(f"w1c{l + 1}root", gam1[l - 1])
                    bias = fold_bias(f"w1c{l + 1}r", f"w1c{l + 1}root",
                                     f"b1c{l + 1}", bet1[l - 1])
                for pr in range(NPR):
                    cols = slice(pr * 2 * N1, (pr + 1) * 2 * N1)
                    agg = sb.tile([128, 2 * N1], F16, tag="aggS", name="aggS")
                    if l == 0:
                        xfp = stp.tile([F0, 2 * N1], F16, tag="xfp", name="xfp")
                        nc.gpsimd.dma_start(xfp[:], x_fm[:, pr * 2 * N1:(pr + 1) * 2 * N1])
                    for gg in range(2):
                        g = 2 * pr + gg
                        aps = psA.tile([128, N1], F32, space="PSUM", tag="A", name="A")
                        if l == 0:
                            xn = stp.tile([128, 2, F0], F16, tag="xn", name="xn")
                            nc.gpsimd.dma_start(xn[:], x_nm_r[g])
                        for vh in range(2):
                            lhs = (xn[:, vh, :] if l == 0
                                   else xr1[l - 1][:, g, vh, :])
                            nc.tensor.matmul(aps[:], lhsT=lhs, rhs=A1[:, g, vh, :],
                                             start=(vh == 0), stop=(vh == 1))
                        nc.vector.tensor_copy(agg[:, gg * N1:(gg + 1) * N1], aps[:])
                    zps = psA.tile([128, 2 * N1], F32, space="PSUM", tag="A", name="A")
                    rhs1 = (xfp[:] if l == 0 else r1[l - 1][:, cols])
                    nc.tensor.matmul(zps[:], lhsT=wr[:], rhs=agg[:], start=True, stop=False)
                    nc.tensor.matmul(zps[:], lhsT=wroot[:], rhs=rhs1, start=False, stop=True)
                    nc.scalar.activation(r1[l][:, cols], zps[:], AF.Relu,
                                         bias=bias[:], accum_out=sum1[l][:, pr:pr + 1])
                    sqs = sb.tile([128, 2 * N1], F16, tag="sqs", name="sqs")
                    nc.scalar.activation(sqs[:], r1[l][:, cols], AF.Square,
                                         accum_out=sq1[l][:, pr:pr + 1])
                for g in range(NG):
                    for uh in range(2):
                        tps = psT.tile([128, ST], F16, space="PSUM", tag="T", name="T")
                        nc.tensor.transpose(
                            tps[:], r1[l][:, g * N1 + uh * 128: g * N1 + (uh + 1) * 128],
                            ident16[:])
                        nc.scalar.copy(xr1[l][:, g, uh, :], tps[:])
                gam1[l], bet1[l], brow1[l] = bn_sync(
                    f"p1l{l}", ST, sum1[l], sq1[l], NT1, f"bn1g{l + 1}", f"bn1b{l + 1}")

            # ---------------- lin1 -> softmax -> s1 ----------------
            wlinS = [scale_w(f"wlin1_{i}", gam1[i][0:64]) for i in range(3)]
            lb_ps = psS.tile([K1, 1], F32, space="PSUM", tag="S", name="S")
            for i in range(3):
                nc.tensor.matmul(lb_ps[:], lhsT=wt[f"wlin1_{i}"][:],
                                 rhs=bet1[i][0:64], start=(i == 0), stop=(i == 2))
            blin = sm.tile([K1, 1], F32, tag="blin", name="blin")
            nc.vector.tensor_tensor(out=blin[:], in0=lb_ps[:], in1=wt["blin1"][:], op=OP.add)

            for g in range(NG):
                sl_ps = psA.tile([K1, N1], F32, space="PSUM", tag="A", name="A")
                for i in range(3):
                    nc.tensor.matmul(sl_ps[:], lhsT=wlinS[i][:],
                                     rhs=r1[i][0:64, g * N1:(g + 1) * N1],
                                     start=(i == 0), stop=(i == 2))
                slin = sb.tile([K1, N1], F16, tag="slin", name="slin")
                nc.scalar.activation(slin[:], sl_ps[:], AF.Relu, bias=blin[:])
                for uh in range(2):
                    tps = psT.tile([128, K1], F16, space="PSUM", tag="T", name="T")
                    nc.tensor.transpose(tps[:], slin[:, uh * 128:(uh + 1) * 128],
                                        ident16[:K1, :K1])
                    mx = sm.tile([128, 1], F32, tag="smx", name="smx")
                    nc.vector.tensor_reduce(mx[:], tps[:], AX.X, OP.max)
                    nc.vector.tensor_scalar_mul(mx[:], mx[:], -1.0)
                    snm = sm.tile([128, K1], F32, tag="snm", name="snm")
                    sme = sm.tile([128, 1], F32, tag="sme", name="sme")
                    nc.scalar.activation(snm[:], tps[:], AF.Exp, bias=mx[:],
                                         accum_out=sme[:])
                    nc.vector.reciprocal(sme[:], sme[:])
                    nc.vector.tensor_scalar_mul(s1[:, g, uh, :], snm[:], sme[:])
                cs_ps = psS.tile([1, K1], F32, space="PSUM", tag="S", name="S")
                for uh in range(2):
                    nc.tensor.matmul(cs_ps[:], lhsT=ones_col[:], rhs=s1[:, g, uh, :],
                                     start=(uh == 0), stop=(uh == 1))
                nc.vector.tensor_copy(cs1[:, g * K1:(g + 1) * K1], cs_ps[:])

            # ---------------- diff_pool 1 ----------------
            for g in range(NG):
                mps = psA.tile([K1, N1], F32, space="PSUM", tag="A", name="A")
                for vh in range(2):
                    nc.tensor.matmul(mps[:], lhsT=s1[:, g, vh, :], rhs=A1[:, g, vh, :],
                                     start=(vh == 0), stop=(vh == 1))
                msb = sb.tile([K1, N1], F16, tag="msb", name="msb")
                nc.vector.tensor_copy(msb[:], mps[:])
                n2t = sm.tile([128, 2, K1], F16, tag="n2t", name="n2t")
                for uh in range(2):
                    tps = psT.tile([128, K1], F16, space="PSUM", tag="T", name="T")
                    nc.tensor.transpose(tps[:], msb[:, uh * 128:(uh + 1) * 128],
                                        ident16[:K1, :K1])
                    nc.vector.tensor_scalar_mul(n2t[:, uh, :], tps[:],
                                                t1col[:, g, uh:uh + 1])
                oa_ps = psS.tile([K1, K1], F32, space="PSUM", tag="S", name="S")
                for uh in range(2):
                    nc.tensor.matmul(oa_ps[:], lhsT=n2t[:, uh, :], rhs=s1[:, g, uh, :],
                                     start=(uh == 0), stop=(uh == 1))
                nc.vector.tensor_copy(A2[:, g * N2:(g + 1) * N2], oa_ps[:])
                for j in range(3):
                    hp = psS.tile([H, K1], F32, space="PSUM", tag="S", name="S")
                    for uh in range(2):
                        nc.tensor.matmul(hp[:], lhsT=xr1[j][:, g, uh, 64:128],
                                         rhs=s1[:, g, uh, :],
                                         start=(uh == 0), stop=(uh == 1))
                    rk = psT.tile([H, K1], F32, space="PSUM", tag="T", name="T")
                    nc.tensor.matmul(rk[:], lhsT=brow1[j][:, 64:128],
                                     rhs=cs1[:, g * K1:(g + 1) * K1], start=True, stop=True)
                    dst = (h1a[64 * j:64 * (j + 1), g * N2:(g + 1) * N2] if j < 2
                           else h1b[:, g * N2:(g + 1) * N2])
                    tmp = sm.tile([H, K1], F32, tag="hfin", name="hfin")
                    nc.scalar.activation(tmp[:], hp[:], AF.Copy,
                                         scale=gam1[j][64:128, 0:1])
                    nc.vector.tensor_tensor(out=dst, in0=tmp[:], in1=rk[:], op=OP.add)
                for (src, c0, c1) in ((h1a, 0, 128), (h1b, 128, 192)):
                    cw = c1 - c0
                    tps = psT.tile([N2, 128], F16, space="PSUM", tag="T", name="T")
                    nc.tensor.transpose(tps[:, :cw], src[:cw, g * N2:(g + 1) * N2],
                                        ident16[:cw, :cw])
                    nc.scalar.copy(h1n[:, g, c0:c1], tps[:, :cw])
                d2r = psS.tile([1, N2], F32, space="PSUM", tag="S", name="S")
                nc.tensor.matmul(d2r[:], lhsT=ones_col[:N2], rhs=A2[:, g * N2:(g + 1) * N2],
                                 start=True, stop=True)
                t2r = sm.tile([1, N2], F16, tag="t2r", name="t2r")
                nc.vector.tensor_scalar_max(t2r[:], d2r[:], 1.0)
                d2c = psS.tile([N2, 1], F32, space="PSUM", tag="S", name="S")
                nc.tensor.matmul(d2c[:], lhsT=A2[:, g * N2:(g + 1) * N2],
                                 rhs=ones_col[:N2], start=True, stop=True)
                nc.vector.tensor_scalar_max(t2col[:, g:g + 1], d2c[:], 1.0)
                b2ps = psT.tile([N2, N2], F32, space="PSUM", tag="T", name="T")
                nc.tensor.matmul(b2ps[:], lhsT=ones_row[:, :N2], rhs=t2r[:],
                                 start=True, stop=True)
                rdb2 = sm.tile([N2, N2], F32, tag="rdb2", name="rdb2")
                nc.vector.reciprocal(rdb2[:], b2ps[:])
                nc.vector.tensor_tensor(out=A2[:, g * N2:(g + 1) * N2],
                                        in0=A2[:, g * N2:(g + 1) * N2],
                                        in1=rdb2[:], op=OP.mult)

            # ======================== PHASE 2 ========================
            p1.release()
            p23 = tc.alloc_tile_pool(name="p23", bufs=1)
            agg2a = p23.tile([128, NG * N2], F16, tag="agg2a", name="agg2a")
            agg2b = p23.tile([64, NG * N2], F16, tag="agg2b", name="agg2b")
            r2 = [p23.tile([ST2L[l], NG * N2], F16, tag=f"r2_{l}", name=f"r2_{l}") for l in range(3)]
            xr2 = [p23.tile([N2, NG, ST2L[l]], F16, tag=f"xr2_{l}", name=f"xr2_{l}") for l in range(3)]
            s2 = p23.tile([N2, NG * K2], F16, tag="s2", name="s2")
            cs2 = p23.tile([1, NG * K2], F16, tag="cs2", name="cs2")
            h2a = p23.tile([128, NG * N3], F16, tag="h2a", name="h2a")
            h2b = p23.tile([64, NG * N3], F16, tag="h2b", name="h2b")
            A3 = p23.tile([N3, NG * N3], F16, tag="A3", name="A3")
            t3col = p23.tile([N3, NG], F32, tag="t3col", name="t3col")
            h2n = p23.tile([N3, NG, 3 * H], F16, tag="h2n", name="h2n")
            agg3a = p23.tile([128, NG * N3], F16, tag="agg3a", name="agg3a")
            agg3b = p23.tile([64, NG * N3], F16, tag="agg3b", name="agg3b")
            r3 = [p23.tile([H, NG * N3], F16, tag=f"r3_{l}", name=f"r3_{l}") for l in range(3)]
            xr3 = [p23.tile([N3, NG, H], F16, tag=f"xr3_{l}", name=f"xr3_{l}") for l in range(3)]
            embA = p23.tile([128, NG], F16, tag="embA", name="embA")
            embB = p23.tile([64, NG], F16, tag="embB", name="embB")
            gam2, bet2, brow2 = [None] * 3, [None] * 3, [None] * 3
            for l in range(3):
                if l == 0:
                    for g in range(NG):
                        pa = psA.tile([128, N2], F32, space="PSUM", tag="A", name="A")
                        pb = psT.tile([64, N2], F32, space="PSUM", tag="T", name="T")
                        nc.tensor.matmul(pa[:], lhsT=h1n[:, g, 0:128],
                                         rhs=A2[:, g * N2:(g + 1) * N2],
                                         start=True, stop=True)
                        nc.tensor.matmul(pb[:], lhsT=h1n[:, g, 128:192],
                                         rhs=A2[:, g * N2:(g + 1) * N2],
                                         start=True, stop=True)
                        nc.vector.tensor_copy(agg2a[:, g * N2:(g + 1) * N2], pa[:])
                        nc.vector.tensor_copy(agg2b[:, g * N2:(g + 1) * N2], pb[:])
                else:
                    wr = scale_w(f"w2c{l + 1}r", gam2[l - 1])
                    wroot = scale_w(f"w2c{l + 1}root", gam2[l - 1])
                    bias = fold_bias(f"w2c{l + 1}r", f"w2c{l + 1}root",
                                     f"b2c{l + 1}", bet2[l - 1])
                    for g in range(NG):
                        pa = psA.tile([128, N2], F32, space="PSUM", tag="A", name="A")
                        nc.tensor.matmul(pa[:], lhsT=xr2[l - 1][:, g, :],
                                         rhs=A2[:, g * N2:(g + 1) * N2],
                                         start=True, stop=True)
                        nc.vector.tensor_copy(agg2a[:, g * N2:(g + 1) * N2], pa[:])
                C2 = ST2L[l]
                for gr in range(NGR2):
                    cols = slice(gr * G2W * N2, (gr + 1) * G2W * N2)
                    zps = psA.tile([C2, G2W * N2], F32, space="PSUM", tag="A", name="A")
                    if l == 0:
                        nc.tensor.matmul(zps[:], lhsT=wt["w2c1r_a"][:], rhs=agg2a[:, cols],
                                         start=True, stop=False)
                        nc.tensor.matmul(zps[:], lhsT=wt["w2c1r_b"][:], rhs=agg2b[:, cols],
                                         start=False, stop=False)
                        nc.tensor.matmul(zps[:], lhsT=wt["w2c1root_a"][:], rhs=h1a[:, cols],
                                         start=False, stop=False)
                        nc.tensor.matmul(zps[:], lhsT=wt["w2c1root_b"][:], rhs=h1b[:, cols],
                                         start=False, stop=True)
                        bias = wt["b2c1"]
                    else:
                        nc.tensor.matmul(zps[:], lhsT=wr[:], rhs=agg2a[:, cols],
                                         start=True, stop=False)
                        nc.tensor.matmul(zps[:], lhsT=wroot[:], rhs=r2[l - 1][:, cols],
                                         start=False, stop=True)
                    nc.scalar.activation(r2[l][:, cols], zps[:], AF.Relu,
                                         bias=bias[:], accum_out=sum2[l][:, gr:gr + 1])
                    sqs = sb.tile([128, 2 * N1], F16, tag="sqs", name="sqs")
                    nc.scalar.activation(sqs[:C2, :G2W * N2], r2[l][:, cols], AF.Square,
                                         accum_out=sq2[l][:, gr:gr + 1])
                for g in range(NG):
                    tps = psT.tile([N2, 128], F16, space="PSUM", tag="T", name="T")
                    nc.tensor.transpose(tps[:, :C2], r2[l][:, g * N2:(g + 1) * N2],
                                        ident16[:C2, :C2])
                    nc.scalar.copy(xr2[l][:, g, :], tps[:, :C2])
                gam2[l], bet2[l], brow2[l] = bn_sync(
                    f"p2l{l}", C2, sum2[l], sq2[l], NT2, f"bn2g{l + 1}", f"bn2b{l + 1}")

            # ---------------- lin2 -> s2 ----------------
            wlin2S = [scale_w(f"wlin2_{i}", gam2[i][0:PW2[i]]) for i in range(3)]
            lb2_ps = psS.tile([K2, 1], F32, space="PSUM", tag="S", name="S")
            for i in range(3):
                nc.tensor.matmul(lb2_ps[:], lhsT=wt[f"wlin2_{i}"][:],
                                 rhs=bet2[i][0:PW2[i]], start=(i == 0), stop=(i == 2))
            blin2 = sm.tile([K2, 1], F32, tag="blin2", name="blin2")
            nc.vector.tensor_tensor(out=blin2[:], in0=lb2_ps[:], in1=wt["blin2"][:], op=OP.add)
            for gr in range(NGR2):
                cols = slice(gr * G2W * N2, (gr + 1) * G2W * N2)
                sl_ps = psA.tile([K2, G2W * N2], F32, space="PSUM", tag="A", name="A")
                for i in range(3):
                    nc.tensor.matmul(sl_ps[:], lhsT=wlin2S[i][:],
                                     rhs=r2[i][0:PW2[i], cols], start=(i == 0), stop=(i == 2))
                slin2 = sb.tile([K2, G2W * N2], F16, tag="slin2", name="slin2")
                nc.scalar.activation(slin2[:], sl_ps[:], AF.Relu, bias=blin2[:])
                for gg in range(G2W):
                    g = gr * G2W + gg
                    tps = psT.tile([N2, K2], F16, space="PSUM", tag="T", name="T")
                    nc.tensor.transpose(tps[:], slin2[:, gg * N2:(gg + 1) * N2],
                                        ident16[:K2, :K2])
                    mx = sm.tile([N2, 1], F32, tag="s2mx", name="s2mx")
                    nc.vector.tensor_reduce(mx[:], tps[:], AX.X, OP.max)
                    nc.vector.tensor_scalar_mul(mx[:], mx[:], -1.0)
                    sme = sm.tile([N2, 1], F32, tag="s2me", name="s2me")
                    se = sm.tile([N2, K2], F32, tag="s2e", name="s2e")
                    nc.scalar.activation(se[:], tps[:], AF.Exp, bias=mx[:], accum_out=sme[:])
                    nc.vector.reciprocal(sme[:], sme[:])
                    nc.vector.tensor_scalar_mul(s2[:, g * K2:(g + 1) * K2], se[:], sme[:])
                    cs_ps = psS.tile([1, K2], F32, space="PSUM", tag="S", name="S")
                    nc.tensor.matmul(cs_ps[:], lhsT=ones_col[:N2],
                                     rhs=s2[:, g * K2:(g + 1) * K2], start=True, stop=True)
                    nc.vector.tensor_copy(cs2[:, g * K2:(g + 1) * K2], cs_ps[:])

            # ---------------- diff_pool 2 ----------------
            # gam2[2] embed slice starts at partition 16 (not 32-aligned);
            # DMA it to a base-0 tile for use as an ACT scale operand.
            gam2e2 = sm.tile([64, 1], F32, tag="gam2e2", name="gam2e2")
            nc.sync.dma_start(gam2e2[:], gam2[2][16:80, 0:1])
            for g in range(NG):
                gsl = slice(g * K2, (g + 1) * K2)
                nsl = slice(g * N2, (g + 1) * N2)
                n3sl = slice(g * N3, (g + 1) * N3)
                mps = psS.tile([K2, N2], F32, space="PSUM", tag="S", name="S")
                nc.tensor.matmul(mps[:], lhsT=s2[:, gsl], rhs=A2[:, nsl],
                                 start=True, stop=True)
                msb = sm.tile([K2, N2], F16, tag="m2sb", name="m2sb")
                nc.vector.tensor_copy(msb[:], mps[:])
                tps = psT.tile([N2, K2], F16, space="PSUM", tag="T", name="T")
                nc.tensor.transpose(tps[:], msb[:], ident16[:K2, :K2])
                n22 = sm.tile([N2, K2], F16, tag="n22", name="n22")
                nc.vector.tensor_scalar_mul(n22[:], tps[:], t2col[:, g:g + 1])
                oa_ps = psS.tile([K2, K2], F32, space="PSUM", tag="S", name="S")
                nc.tensor.matmul(oa_ps[:], lhsT=n22[:], rhs=s2[:, gsl], start=True, stop=True)
                nc.vector.tensor_copy(A3[:, n3sl], oa_ps[:])
                for j in range(3):
                    hp = psS.tile([H, K2], F32, space="PSUM", tag="S", name="S")
                    nc.tensor.matmul(hp[:], lhsT=xr2[j][:, g, PW2[j]:ST2L[j]],
                                     rhs=s2[:, gsl], start=True, stop=True)
                    rk = psT.tile([H, K2], F32, space="PSUM", tag="T", name="T")
                    nc.tensor.matmul(rk[:], lhsT=brow2[j][:, PW2[j]:ST2L[j]],
                                     rhs=cs2[:, gsl], start=True, stop=True)
                    dst = (h2a[64 * j:64 * (j + 1), n3sl] if j < 2 else h2b[:, n3sl])
                    tmp = sm.tile([H, K2], F32, tag="h2fin", name="h2fin")
                    scl = gam2e2[:, 0:1] if j == 2 else gam2[j][64:128, 0:1]
                    nc.scalar.activation(tmp[:], hp[:], AF.Copy, scale=scl)
                    nc.vector.tensor_tensor(out=dst, in0=tmp[:], in1=rk[:], op=OP.add)
                for (src, c0, c1) in ((h2a, 0, 128), (h2b, 128, 192)):
                    cw = c1 - c0
                    tps2 = psT.tile([N3, 128], F16, space="PSUM", tag="T", name="T")
                    nc.tensor.transpose(tps2[:, :cw], src[:cw, n3sl], ident16[:cw, :cw])
                    nc.scalar.copy(h2n[:, g, c0:c1], tps2[:, :cw])
                d3r = psS.tile([1, N3], F32, space="PSUM", tag="S", name="S")
                nc.tensor.matmul(d3r[:], lhsT=ones_col[:N3], rhs=A3[:, n3sl],
                                 start=True, stop=True)
                t3r = sm.tile([1, N3], F16, tag="t3r", name="t3r")
                nc.vector.tensor_scalar_max(t3r[:], d3r[:], 1.0)
                d3c = psS.tile([N3, 1], F32, space="PSUM", tag="S", name="S")
                nc.tensor.matmul(d3c[:], lhsT=A3[:, n3sl], rhs=ones_col[:N3],
                                 start=True, stop=True)
                nc.vector.tensor_scalar_max(t3col[:, g:g + 1], d3c[:], 1.0)
                b3ps = psT.tile([N3, N3], F32, space="PSUM", tag="T", name="T")
                nc.tensor.matmul(b3ps[:], lhsT=ones_row[:, :N3], rhs=t3r[:],
                                 start=True, stop=True)
                rdb3 = sm.tile([N3, N3], F32, tag="rdb3", name="rdb3")
                nc.vector.reciprocal(rdb3[:], b3ps[:])
                nc.vector.tensor_tensor(out=A3[:, n3sl], in0=A3[:, n3sl],
                                        in1=rdb3[:], op=OP.mult)

            # ======================== PHASE 3 ========================
            gam3, bet3 = [None] * 3, [None] * 3
            for l in range(3):
                if l == 0:
                    for g in range(NG):
                        n3sl = slice(g * N3, (g + 1) * N3)
                        pa = psA.tile([128, N3], F32, space="PSUM", tag="A", name="A")
                        pb = psT.tile([64, N3], F32, space="PSUM", tag="T", name="T")
                        nc.tensor.matmul(pa[:], lhsT=h2n[:, g, 0:128], rhs=A3[:, n3sl],
                                         start=True, stop=True)
                        nc.tensor.matmul(pb[:], lhsT=h2n[:, g, 128:192], rhs=A3[:, n3sl],
                                         start=True, stop=True)
                        nc.vector.tensor_copy(agg3a[:, n3sl], pa[:])
                        nc.vector.tensor_copy(agg3b[:, n3sl], pb[:])
                    zps = psA.tile([H, NG * N3], F32, space="PSUM", tag="A", name="A")
                    nc.tensor.matmul(zps[:], lhsT=wt["w3c1r_a"][:], rhs=agg3a[:],
                                     start=True, stop=False)
                    nc.tensor.matmul(zps[:], lhsT=wt["w3c1r_b"][:], rhs=agg3b[:],
                                     start=False, stop=False)
                    nc.tensor.matmul(zps[:], lhsT=wt["w3c1root_a"][:], rhs=h2a[:],
                                     start=False, stop=False)
                    nc.tensor.matmul(zps[:], lhsT=wt["w3c1root_b"][:], rhs=h2b[:],
                                     start=False, stop=True)
                    bias = wt["b3c1"]
                else:
                    wr = scale_w(f"w3c{l + 1}r", gam3[l - 1])
                    wroot = scale_w(f"w3c{l + 1}root", gam3[l - 1])
                    bias = fold_bias(f"w3c{l + 1}r", f"w3c{l + 1}root",
                                     f"b3c{l + 1}", bet3[l - 1])
                    for g in range(NG):
                        n3sl = slice(g * N3, (g + 1) * N3)
                        pa = psT.tile([H, N3], F32, space="PSUM", tag="T", name="T")
                        nc.tensor.matmul(pa[:], lhsT=xr3[l - 1][:, g, :], rhs=A3[:, n3sl],
                                         start=True, stop=True)
                        nc.vector.tensor_copy(agg3a[0:H, n3sl], pa[:])
                    zps = psA.tile([H, NG * N3], F32, space="PSUM", tag="A", name="A")
                    nc.tensor.matmul(zps[:], lhsT=wr[:], rhs=agg3a[0:H, :],
                                     start=True, stop=False)
                    nc.tensor.matmul(zps[:], lhsT=wroot[:], rhs=r3[l - 1][:],
                                     start=False, stop=True)
                nc.scalar.activation(r3[l][:], zps[:], AF.Relu, bias=bias[:],
                                     accum_out=sum3[l][:, 0:1])
                sqs = sb.tile([128, 2 * N1], F16, tag="sqs", name="sqs")
                nc.scalar.activation(sqs[0:H, :NG * N3], r3[l][:], AF.Square,
                                     accum_out=sq3[l][:, 0:1])
                for g in range(NG):
                    tps = psT.tile([N3, H], F16, space="PSUM", tag="T", name="T")
                    nc.tensor.transpose(tps[:], r3[l][:, g * N3:(g + 1) * N3],
                                        ident16[:H, :H])
                    nc.scalar.copy(xr3[l][:, g, :], tps[:])
                gam3[l], bet3[l], _ = bn_sync(
                    f"p3l{l}", H, sum3[l], sq3[l], NT3, f"bn3g{l + 1}", f"bn3b{l + 1}")

            # ---------------- emb -> lin1 -> lin2 -> log_softmax ----------------
            for j in range(3):
                t = sm.tile([H, NG], F32, tag="embred", name="embred")
                nc.vector.tensor_reduce(
                    t[:], r3[j][:].rearrange("c (g u) -> c g u", g=NG), AX.X, OP.add)
                g16 = sm.tile([H, 1], F32, tag="g16", name="g16")
                nc.vector.tensor_scalar_mul(g16[:], gam3[j][:], 1.0 / N3)
                dst = embA[64 * j:64 * (j + 1), :] if j < 2 else embB[:, :]
                nc.vector.scalar_tensor_tensor(
                    out=dst, in0=t[:], scalar=g16[:],
                    in1=bet3[j][:, 0:1].to_broadcast((H, NG)),
                    op0=OP.mult, op1=OP.add)
            l1ps = psS.tile([H, NG], F32, space="PSUM", tag="S", name="S")
            nc.tensor.matmul(l1ps[:], lhsT=wt["wlin1f_a"][:], rhs=embA[:],
                             start=True, stop=False)
            nc.tensor.matmul(l1ps[:], lhsT=wt["wlin1f_b"][:], rhs=embB[:],
                             start=False, stop=True)
            hf = sm.tile([H, NG], F16, tag="hf", name="hf")
            nc.scalar.activation(hf[:], l1ps[:], AF.Relu, bias=wt["blin1f"][:])
            l2ps = psS.tile([NCLS, NG], F32, space="PSUM", tag="S", name="S")
            nc.tensor.matmul(l2ps[:], lhsT=wt["wlin2f"][:], rhs=hf[:], start=True, stop=True)
            lg = sm.tile([NCLS, NG], F32, tag="lg", name="lg")
            nc.scalar.activation(lg[:], l2ps[:], AF.Identity, bias=wt["blin2f"][:])
            lgt_ps = psS.tile([NG, NCLS], F32, space="PSUM", tag="S", name="S")
            nc.tensor.transpose(lgt_ps[:], lg[:], ident32[:NCLS, :NCLS])
            lgt = sm.tile([NG, NCLS], F32, tag="lgt", name="lgt")
            nc.vector.tensor_copy(lgt[:], lgt_ps[:])
            mx = sm.tile([NG, 1], F32, tag="lmx", name="lmx")
            nc.vector.tensor_reduce(mx[:], lgt[:], AX.X, OP.max)
            nc.vector.tensor_scalar_mul(mx[:], mx[:], -1.0)
            esum = sm.tile([NG, 1], F32, tag="lesum", name="lesum")
            etmp = sm.tile([NG, NCLS], F32, tag="letmp", name="letmp")
            nc.scalar.activation(etmp[:], lgt[:], AF.Exp, bias=mx[:], accum_out=esum[:])
            lse = sm.tile([NG, 1], F32, tag="llse", name="llse")
            nc.scalar.activation(lse[:], esum[:], AF.Ln)
            outt = sm.tile([NG, NCLS], F32, tag="outt", name="outt")
            nc.vector.tensor_scalar(out=outt[:], in0=lgt[:], scalar1=mx[:],
                                    scalar2=lse[:], op0=OP.add, op1=OP.subtract)
            nc.sync.dma_start(out_d[:, :], outt[:])
            p23.release()

    nc.compile()
    return nc


def kernel(x, batch, edge_index, params):
    in_maps, nidx, wshapes = _prep(x, batch, edge_index, params)
    key = (nidx, tuple(sorted(wshapes.items())))
    if key not in _CACHE:
        _CACHE[key] = _build(nidx, wshapes)
    nc = _CACHE[key]
    res = run_bass_kernel_spmd(nc, in_maps, core_ids=list(range(N_CORES)))
    return np.concatenate([r["out"] for r in res.results], axis=0)


# revision 24
# speedup vs baseline: 6567.1501x; 6567.1501x over previous
"""DiffPoolNet on 8 TRN2 NeuronCores (Bass/Tile).

Sharding: data-parallel over graphs, 32 graphs per core. Per graph the dense
256x256 adjacency (transposed, degree-normalized) lives in SBUF as fp16,
built on-device by the GPSIMD local_scatter instruction from host-bucketed
edge index lists -- the dense adjacency never touches HBM.

Activations are feature-major ([channels, nodes]); the pool/embed GNN blocks
are stacked on the partition axis (pool = 0:64, embed = 64:128) so one matmul
with block-diagonal weights serves both blocks.

Training-mode BatchNorm needs global stats: each of the 9 BN layers does a
[<=128,2] AllReduce of (sum, sumsq) across the 8 cores. The BN affine is
folded into the *next* layer's weights/bias (gamma-scaled weights, bias
absorbing the beta terms), so the heavy per-graph adjacency matmuls of layer
l+1 depend only on pre-BN activations and can overlap the collective.

All matmul operands are fp16 (fp32 PSUM accumulate); measured end-to-end
error vs the fp32 reference is ~8e-4 max-rel.
"""
import numpy as np

import concourse.bacc as bacc
import concourse.mybir as mybir
import concourse.tile as tile
from concourse import library_config
from concourse.bass_utils import run_bass_kernel_spmd

F32 = mybir.dt.float32
F16 = mybir.dt.float16
I16 = mybir.dt.int16
AF = mybir.ActivationFunctionType
OP = mybir.AluOpType
AX = mybir.AxisListType

import os
N_CORES = int(os.environ.get("KNC", "8"))
KSIM = os.environ.get("KSIM", "0") == "1"
B = 256
NG = int(os.environ.get("KNG", str(B // N_CORES)))    # graphs per core
N1, F0, H = 256, 128, 64
ST = 2 * H
K1, K2, N2, N3 = 64, 16, 64, 16
NCLS = 10
EPS = 1e-5
NT1, NT2, NT3 = B * N1, B * N2, B * N3

_CACHE = {}


# --------------------------------------------------------------------------
# Host prep
# --------------------------------------------------------------------------

def _prep(x, batch, edge_index, params):
    x = np.asarray(x)
    batch = np.asarray(batch)
    e0, e1 = np.asarray(edge_index)

    counts = np.bincount(batch, minlength=B)
    assert counts.shape[0] == B and np.all(counts == N1), "expects 256 nodes/graph"
    starts = np.concatenate([[0], np.cumsum(counts)[:-1]]).astype(np.int64)
    pos = np.arange(batch.shape[0], dtype=np.int64) - starts[batch]

    g = batch[e0].astype(np.int64)
    u = pos[e0]
    v = pos[e1]
    # one sort: key = (g, v%128, elem) -- bucket-major, duplicates adjacent
    p = v & 127
    elem = ((v >> 7) << 8) + u          # scatter target in A^T [128, 2, 256]
    bucket = g * 128 + p
    key = (bucket << 9) | elem
    key.sort()
    key = key[np.concatenate([[True], key[1:] != key[:-1]])]   # dedup (g,u,v)
    bucket_s = key >> 9
    elem_s = key & 0x1FF

    deg = np.bincount(
        (bucket_s >> 7) * N1 + (elem_s & 0xFF), minlength=B * N1)
    assert deg.min() >= 1, "zero out-degree node: rank-1 delta path not built"

    bc = np.bincount(bucket_s, minlength=B * 128)
    bstart = np.concatenate([[0], np.cumsum(bc)])
    rank = np.arange(elem_s.shape[0]) - bstart[bucket_s]
    nidx = max(64, (int(bc.max()) + 1) & ~1)
    assert nidx <= 512

    idx_all = np.full((B * 128, nidx), -1, np.int16)
    idx_all[bucket_s, rank] = elem_s.astype(np.int16)
    idx_all = idx_all.reshape(B, 128, nidx)

    P = params
    W = {}

    def cat_T(a, b):
        return np.concatenate([np.asarray(a).T, np.asarray(b).T], axis=1).astype(np.float32)

    def blk_T(a, b):
        a, b = np.asarray(a), np.asarray(b)
        fia, foa = a.shape[1], a.shape[0]
        fib, fob = b.shape[1], b.shape[0]
        w = np.zeros((fia + fib, foa + fob), np.float32)
        w[:fia, :foa] = a.T
        w[fia:, foa:] = b.T
        return w

    def cat_v(a, b):
        return np.concatenate([np.asarray(a), np.asarray(b)]).astype(np.float32)[:, None]

    def col(a):
        return np.asarray(a).astype(np.float32)[:, None]

    for ph, pp_, pe_ in (("1", "gnn1_pool", "gnn1_embed"),
                         ("2", "gnn2_pool", "gnn2_embed")):
        cp, ce = P[pp_], P[pe_]
        c1r = cat_T(cp["conv1"]["Wr"], ce["conv1"]["Wr"])        # [fi, 128]
        c1o = cat_T(cp["conv1"]["Wroot"], ce["conv1"]["Wroot"])
        if ph == "2":   # fi = 192 > 128: split partition chunks
            W["w2c1r_a"], W["w2c1r_b"] = c1r[:128], c1r[128:]
            W["w2c1root_a"], W["w2c1root_b"] = c1o[:128], c1o[128:]
        else:
            W["w1c1r"], W["w1c1root"] = c1r, c1o
        W[f"b{ph}c1"] = cat_v(cp["conv1"]["br"], ce["conv1"]["br"])
        for l in (2, 3):
            W[f"w{ph}c{l}r"] = blk_T(cp[f"conv{l}"]["Wr"], ce[f"conv{l}"]["Wr"])
            W[f"w{ph}c{l}root"] = blk_T(cp[f"conv{l}"]["Wroot"], ce[f"conv{l}"]["Wroot"])
            W[f"b{ph}c{l}"] = cat_v(cp[f"conv{l}"]["br"], ce[f"conv{l}"]["br"])
        for l in (1, 2, 3):
            W[f"bn{ph}g{l}"] = cat_v(cp[f"bn{l}"]["g"], ce[f"bn{l}"]["g"])
            W[f"bn{ph}b{l}"] = cat_v(cp[f"bn{l}"]["b"], ce[f"bn{l}"]["b"])
        lw = np.asarray(cp["lin"]["W"]).T.astype(np.float32)
        pw = 64 if ph == "1" else 16    # pool conv3 output width
        csz = [64, 64, pw]
        off = 0
        for i in range(3):
            W[f"wlin{ph}_{i}"] = lw[off:off + csz[i]]
            off += csz[i]
        W[f"blin{ph}"] = col(cp["lin"]["b"])

    c3 = P["gnn3_embed"]
    w = np.asarray(c3["conv1"]["Wr"]).T.astype(np.float32)
    W["w3c1r_a"], W["w3c1r_b"] = w[:128], w[128:]
    w = np.asarray(c3["conv1"]["Wroot"]).T.astype(np.float32)
    W["w3c1root_a"], W["w3c1root_b"] = w[:128], w[128:]
    W["b3c1"] = col(c3["conv1"]["br"])
    for l in (2, 3):
        W[f"w3c{l}r"] = np.asarray(c3[f"conv{l}"]["Wr"]).T.astype(np.float32)
        W[f"w3c{l}root"] = np.asarray(c3[f"conv{l}"]["Wroot"]).T.astype(np.float32)
        W[f"b3c{l}"] = col(c3[f"conv{l}"]["br"])
    for l in (1, 2, 3):
        W[f"bn3g{l}"] = col(c3[f"bn{l}"]["g"])
        W[f"bn3b{l}"] = col(c3[f"bn{l}"]["b"])
    W["ident16"] = np.eye(128, dtype=np.float32)
    W["ident32"] = np.eye(128, dtype=np.float32)
    w = np.asarray(P["lin1"]["W"]).T.astype(np.float32)
    W["wlin1f_a"], W["wlin1f_b"] = w[:128], w[128:]
    W["blin1f"] = col(P["lin1"]["b"])
    W["wlin2f"] = np.asarray(P["lin2"]["W"]).T.astype(np.float32)
    W["blin2f"] = col(P["lin2"]["b"])

    in_maps = []
    for c in range(N_CORES):
        xs = x[c * NG * N1:(c + 1) * NG * N1].astype(np.float32)
        m = {
            "x_nm": np.ascontiguousarray(xs),
            "x_fm": np.ascontiguousarray(xs.T),
            "scat": np.ascontiguousarray(
                idx_all[c * NG:(c + 1) * NG].transpose(1, 0, 2).reshape(128, NG * nidx)),
        }
        m.update(W)
        in_maps.append(m)
    return in_maps, nidx, {k: tuple(w.shape) for k, w in W.items()}


# --------------------------------------------------------------------------
# Device program
# --------------------------------------------------------------------------

def _build(nidx, wshapes):
    nc = bacc.Bacc("TRN2", target_bir_lowering=False, debug=False,
                   num_devices=N_CORES)
    x_nm = nc.dram_tensor("x_nm", [NG * N1, F0], F32, kind="ExternalInput")
    x_fm = nc.dram_tensor("x_fm", [F0, NG * N1], F32, kind="ExternalInput")
    scat = nc.dram_tensor("scat", [128, NG * nidx], I16, kind="ExternalInput")
    wd = {k: nc.dram_tensor(k, list(s), F32, kind="ExternalInput")
          for k, s in wshapes.items()}
    out_d = nc.dram_tensor("out", [NG, NCLS], F32, kind="ExternalOutput")

    RG = [list(range(N_CORES))]

    with tile.TileContext(nc) as tc:
        with (
            tc.tile_pool(name="persist", bufs=1) as pp,
            tc.tile_pool(name="wpool", bufs=1) as wp,
            tc.tile_pool(name="wsp", bufs=1) as wsp,
            tc.tile_pool(name="stream", bufs=2) as stp,
            tc.tile_pool(name="sb", bufs=2) as sb,
            tc.tile_pool(name="sb2", bufs=2) as sb2,
            tc.tile_pool(name="sbr", bufs=1) as sbr,
            tc.tile_pool(name="sbsm", bufs=3) as sm,
            tc.tile_pool(name="stats", bufs=1) as sp,
            tc.tile_pool(name="psA", bufs=3, space="PSUM") as psA,
            tc.tile_pool(name="psT", bufs=3, space="PSUM") as psT,
            tc.tile_pool(name="psS", bufs=2, space="PSUM") as psS,
            tc.tile_pool(name="dram", bufs=1, space="DRAM") as dr,
        ):
            nc.gpsimd.load_library(library_config.local_scatter)

            ones_col = wp.tile([128, 1], F16, tag="ones_col", name="ones_col")
            nc.vector.memset(ones_col[:], 1.0)
            ones_row = wp.tile([1, 128], F16, tag="ones_row", name="ones_row")
            nc.vector.memset(ones_row[:], 1.0)
            ones_dat = wp.tile([128, nidx], F16, tag="ones_dat", name="ones_dat")
            nc.vector.memset(ones_dat[:], 1.0)

            wt = {}
            for k, s in wshapes.items():
                if k[0] == "b" or k == "ident32":   # f32 tiles
                    t = wp.tile(list(s), F32, tag=k, name=k)
                    nc.sync.dma_start(t[:], wd[k][:, :])
                else:
                    t = wp.tile(list(s), F16, tag=k, name=k)
                    nc.gpsimd.dma_start(t[:], wd[k][:, :])
                wt[k] = t
            ident16 = wt["ident16"]
            ident32 = wt["ident32"]

            # ---------------- persistent arrays ----------------
            # pp: survives the phase1->phase2 boundary
            h1a = pp.tile([128, NG * N2], F16, tag="h1a", name="h1a")
            h1b = pp.tile([64, NG * N2], F16, tag="h1b", name="h1b")
            A2 = pp.tile([N2, NG * N2], F16, tag="A2", name="A2")
            t2col = pp.tile([N2, NG], F32, tag="t2col", name="t2col")
            h1n = pp.tile([N2, NG, 3 * H], F16, tag="h1n", name="h1n")
            ST2L = [128, 128, 80]   # phase-2 stacked widths (pool conv3 = 16ch)
            PW2 = [64, 64, 16]      # phase-2 pool widths

            # p1: phase-1 only, released before the phase-2/3 arrays are live
            p1 = tc.alloc_tile_pool(name="p1", bufs=1)
            A1 = p1.tile([128, NG, 2, N1], F16, tag="A1", name="A1")
            t1col = p1.tile([128, NG, 2], F32, tag="t1col", name="t1col")
            r1 = [p1.tile([ST, NG * N1], F16, tag=f"r1_{l}", name=f"r1_{l}") for l in range(3)]
            xr1 = [p1.tile([128, NG, 2, ST], F16, tag=f"xr1_{l}", name=f"xr1_{l}") for l in range(3)]
            s1 = p1.tile([128, NG, 2, K1], F16, tag="s1", name="s1")
            cs1 = p1.tile([1, NG * K1], F16, tag="cs1", name="cs1")

            NPR = NG // 2                 # phase-1 pair count
            G2W = min(8, NG)              # phase-2 graphs per group
            NGR2 = NG // G2W
            sum1 = [sp.tile([ST, NPR], F32, tag=f"sum1_{l}", name=f"sum1_{l}") for l in range(3)]
            sq1 = [sp.tile([ST, NPR], F32, tag=f"sq1_{l}", name=f"sq1_{l}") for l in range(3)]
            sum2 = [sp.tile([ST2L[l], NGR2], F32, tag=f"sum2_{l}", name=f"sum2_{l}") for l in range(3)]
            sq2 = [sp.tile([ST2L[l], NGR2], F32, tag=f"sq2_{l}", name=f"sq2_{l}") for l in range(3)]
            sum3 = [sp.tile([H, 1], F32, tag=f"sum3_{l}", name=f"sum3_{l}") for l in range(3)]
            sq3 = [sp.tile([H, 1], F32, tag=f"sq3_{l}", name=f"sq3_{l}") for l in range(3)]

            x_nm_r = x_nm.ap().rearrange("(g h p) f -> g p h f", g=NG, h=2, p=128)

            # ---------------- adjacency build ----------------
            for g in range(NG):
                sct = stp.tile([128, nidx], I16, tag="sct", name="sct")
                nc.sync.dma_start(sct[:], scat[:, g * nidx:(g + 1) * nidx])
                araw = sb2.tile([128, 2, N1], F16, tag="araw", name="araw")
                nc.gpsimd.local_scatter(
                    out_ap=araw[:], data_ap=ones_dat[:], idxs_ap=sct[:],
                    channels=128, num_elems=2 * N1, num_idxs=nidx)
                dps = psS.tile([1, N1], F32, space="PSUM", tag="S", name="S")
                for vh in range(2):
                    nc.tensor.matmul(dps[:], lhsT=ones_col[:], rhs=araw[:, vh, :],
                                     start=(vh == 0), stop=(vh == 1))
                trow = sm.tile([1, N1], F16, tag="trow", name="trow")
                nc.vector.tensor_scalar_max(trow[:], dps[:], 1.0)
                dcol = psS.tile([128, 2], F32, space="PSUM", tag="S", name="S")
                for uh in range(2):
                    for vh in range(2):
                        nc.tensor.matmul(
                            dcol[:, uh:uh + 1],
                            lhsT=araw[:, vh, uh * 128:(uh + 1) * 128],
                            rhs=ones_col[:], start=(vh == 0), stop=(vh == 1))
                nc.vector.tensor_scalar_max(t1col[:, g, :], dcol[:], 1.0)
                bps = psA.tile([128, 2, N1], F32, space="PSUM", tag="A", name="A")
                for vh in range(2):
                    nc.tensor.matmul(bps[:, vh, :], lhsT=ones_row[:], rhs=trow[:],
                                     start=True, stop=True)
                rdb = sbr.tile([128, 2, N1], F32, tag="rdb", name="rdb")
                nc.vector.reciprocal(rdb[:], bps[:])
                nc.vector.tensor_tensor(out=A1[:, g, :, :], in0=araw[:], in1=rdb[:],
                                        op=OP.mult)

            # ---------------- helpers ----------------
            def bn_sync(tag, C, sumc, sqc, NT, gk, bk):
                stg = sm.tile([C, 2], F32, tag="bn_stg", name="bn_stg")
                if sumc.shape[1] > 1:
                    nc.vector.tensor_reduce(stg[:, 0:1], sumc[:], AX.X, OP.add)
                    nc.vector.tensor_reduce(stg[:, 1:2], sqc[:], AX.X, OP.add)
                else:
                    nc.vector.tensor_copy(stg[:, 0:1], sumc[:])
                    nc.vector.tensor_copy(stg[:, 1:2], sqc[:])
                red = sm.tile([C, 2], F32, tag="bn_red", name="bn_red")
                if KSIM:
                    nc.vector.tensor_copy(red[:], stg[:])
                else:
                    bin_ = dr.tile([C, 2], F32, tag=f"cc_in_{tag}", name=f"cc_in_{tag}")
                    bout = dr.tile([C, 2], F32, tag=f"cc_out_{tag}", name=f"cc_out_{tag}")
                    nc.gpsimd.dma_start(bin_[:], stg[:])
                    nc.gpsimd.collective_compute(
                        "AllReduce", OP.add, replica_groups=RG,
                        ins=[bin_.opt()], outs=[bout.opt()])
                    nc.gpsimd.dma_start(red[:], bout[:])
                mu = sm.tile([C, 1], F32, tag="bn_mu", name="bn_mu")
                nc.vector.tensor_scalar_mul(mu[:], red[:, 0:1], 1.0 / NT)
                var = sm.tile([C, 1], F32, tag="bn_var", name="bn_var")
                nc.vector.tensor_tensor(out=var[:], in0=mu[:], in1=mu[:], op=OP.mult)
                msq = sm.tile([C, 1], F32, tag="bn_msq", name="bn_msq")
                nc.vector.tensor_scalar(out=msq[:], in0=red[:, 1:2], scalar1=1.0 / NT,
                                        scalar2=EPS, op0=OP.mult, op1=OP.add)
                nc.vector.tensor_tensor(out=var[:], in0=msq[:], in1=var[:], op=OP.subtract)
                rec = sm.tile([C, 1], F32, tag="bn_rec", name="bn_rec")
                nc.vector.reciprocal(rec[:], var[:])
                rstd = sm.tile([C, 1], F32, tag="bn_rstd", name="bn_rstd")
                nc.scalar.activation(rstd[:], rec[:], AF.Sqrt)
                gam = sm.tile([C, 1], F32, tag="bn_gam", name="bn_gam")
                nc.vector.tensor_tensor(out=gam[:], in0=wt[gk][:], in1=rstd[:], op=OP.mult)
                bet = sm.tile([C, 1], F32, tag="bn_bet", name="bn_bet")
                nc.vector.tensor_tensor(out=bet[:], in0=mu[:], in1=gam[:], op=OP.mult)
                nc.vector.tensor_tensor(out=bet[:], in0=wt[bk][:], in1=bet[:], op=OP.subtract)
                bet16 = sm.tile([C, 1], F16, tag="bn_bet16", name="bn_bet16")
                nc.vector.tensor_copy(bet16[:], bet[:])
                brps = psS.tile([1, C], F16, space="PSUM", tag="S", name="S")
                nc.tensor.transpose(brps[:], bet16[:], ident16[:C, :C])
                brow = sm.tile([1, C], F16, tag="bn_brow", name="bn_brow")
                nc.vector.tensor_copy(brow[:], brps[:])
                return gam, bet16, brow

            def scale_w(wk, gam):
                w = wt[wk]
                K = w.shape[0]
                ws = wsp.tile(list(w.shape), F16, tag=f"ws_{wk}", name=f"ws_{wk}")
                nc.vector.tensor_tensor(out=ws[:], in0=w[:],
                                        in1=gam[:K, 0:1].to_broadcast(tuple(w.shape)),
                                        op=OP.mult)
                return ws

            def fold_bias(wrk, wrootk, bk, bet16):
                K = wt[wrk].shape[0]
                C = wt[wrk].shape[1]
                ps = psS.tile([C, 1], F32, space="PSUM", tag="S", name="S")
                nc.tensor.matmul(ps[:], lhsT=wt[wrk][:], rhs=bet16[:K], start=True, stop=False)
                nc.tensor.matmul(ps[:], lhsT=wt[wrootk][:], rhs=bet16[:K], start=False, stop=True)
                bias = sm.tile([C, 1], F32, tag="fb_bias", name="fb_bias")
                nc.vector.tensor_tensor(out=bias[:], in0=ps[:], in1=wt[bk][:], op=OP.add)
                return bias

            # ======================== PHASE 1 ========================
            gam1, bet1, brow1 = [None] * 3, [None] * 3, [None] * 3
            for l in range(3):
                if l == 0:
                    wr, wroot, bias = wt["w1c1r"], wt["w1c1root"], wt["b1c1"]
                else:
                    wr = scale_w(f"w1c{l + 1}r", gam1[l - 1])
                    wroot = scale_w(f"w1c{l + 1}root", gam1[l - 1])
                    bias = fold_bias(f"w1c{l + 1}r", f"w1c{l + 1}root",
                                     f"b1c{l + 1}", bet1[l - 1])
                for pr in range(NPR):
                    cols = slice(pr * 2 * N1, (pr + 1) * 2 * N1)
                    agg = sb.tile([128, 2 * N1], F16, tag="aggS", name="aggS")
                    if l == 0:
                        xfp = stp.tile([F0, 2 * N1], F16, tag="xfp", name="xfp")
                        nc.gpsimd.dma_start(xfp[:], x_fm[:, pr * 2 * N1:(pr + 1) * 2 * N1])
                    for gg in range(2):
                        g = 2 * pr + gg
                        aps = psA.tile([128, N1], F32, space="PSUM", tag="A", name="A")
                        if l == 0:
                            xn = stp.tile([128, 2, F0], F16, tag="xn", name="xn")
                            nc.gpsimd.dma_start(xn[:], x_nm_r[g])
                        for vh in range(2):
                            lhs = (xn[:, vh, :] if l == 0
                                   else xr1[l - 1][:, g, vh, :])
                            nc.tensor.matmul(aps[:], lhsT=lhs, rhs=A1[:, g, vh, :],
                                             start=(vh == 0), stop=(vh == 1))
                        nc.vector.tensor_copy(agg[:, gg * N1:(gg + 1) * N1], aps[:])
                    zps = psA.tile([128, 2 * N1], F32, space="PSUM", tag="A", name="A")
                    rhs1 = (xfp[:] if l == 0 else r1[l - 1][:, cols])
                    nc.tensor.matmul(zps[:], lhsT=wr[:], rhs=agg[:], start=True, stop=False)
                    nc.tensor.matmul(zps[:], lhsT=wroot[:], rhs=rhs1, start=False, stop=True)
                    nc.scalar.activation(r1[l][:, cols], zps[:], AF.Relu,
                                         bias=bias[:], accum_out=sum1[l][:, pr:pr + 1])
                    sqs = sb.tile([128, 2 * N1], F16, tag="sqs", name="sqs")
                    nc.scalar.activation(sqs[:], r1[l][:, cols], AF.Square,
                                         accum_out=sq1[l][:, pr:pr + 1])
                for g in range(NG):
                    for uh in range(2):
                        tps = psT.tile([128, ST], F16, space="PSUM", tag="T", name="T")
                        nc.tensor.transpose(
                            tps[:], r1[l][:, g * N1 + uh * 128: g * N1 + (uh + 1) * 128],
                            ident16[:])
                        nc.scalar.copy(xr1[l][:, g, uh, :], tps[:])
                gam1[l], bet1[l], brow1[l] = bn_sync(
                    f"p1l{l}", ST, sum1[l], sq1[l], NT1, f"bn1g{l + 1}", f"bn1b{l + 1}")

            # ---------------- lin1 -> softmax -> s1 ----------------
            wlinS = [scale_w(f"wlin1_{i}", gam1[i][0:64]) for i in range(3)]
            lb_ps = psS.tile([K1, 1], F32, space="PSUM", tag="S", name="S")
            for i in range(3):
                nc.tensor.matmul(lb_ps[:], lhsT=wt[f"wlin1_{i}"][:],
                                 rhs=bet1[i][0:64], start=(i == 0), stop=(i == 2))
            blin = sm.tile([K1, 1], F32, tag="blin", name="blin")
            nc.vector.tensor_tensor(out=blin[:], in0=lb_ps[:], in1=wt["blin1"][:], op=OP.add)

            for g in range(NG):
                sl_ps = psA.tile([K1, N1], F32, space="PSUM", tag="A", name="A")
                for i in range(3):
                    nc.tensor.matmul(sl_ps[:], lhsT=wlinS[i][:],
                                     rhs=r1[i][0:64, g * N1:(g + 1) * N1],
                                     start=(i == 0), stop=(i == 2))
                slin = sb.tile([K1, N1], F16, tag="slin", name="slin")
                nc.scalar.activation(slin[:], sl_ps[:], AF.Relu, bias=blin[:])
                for uh in range(2):
                    tps = psT.tile([128, K1], F16, space="PSUM", tag="T", name="T")
                    nc.tensor.transpose(tps[:], slin[:, uh * 128:(uh + 1) * 128],
                                        ident16[:K1, :K1])
                    mx = sm.tile([128, 1], F32, tag="smx", name="smx")
                    nc.vector.tensor_reduce(mx[:], tps[:], AX.X, OP.max)
                    nc.vector.tensor_scalar_mul(mx[:], mx[:], -1.0)
                    snm = sm.tile([128, K1], F32, tag="snm", name="snm")
                    sme = sm.tile([128, 1], F32, tag="sme", name="sme")
                    nc.scalar.activation(snm[:], tps[:], AF.Exp, bias=mx[:],
                                         accum_out=sme[:])
                    nc.vector.reciprocal(sme[:], sme[:])
                    nc.vector.tensor_scalar_mul(s1[:, g, uh, :], snm[:], sme[:])
                cs_ps = psS.tile([1, K1], F32, space="PSUM", tag="S", name="S")
                for uh in range(2):
                    nc.tensor.matmul(cs_ps[:], lhsT=ones_col[:], rhs=s1[:, g, uh, :],
                                     start=(uh == 0), stop=(uh == 1))
                nc.vector.tensor_copy(cs1[:, g * K1:(g + 1) * K1], cs_ps[:])

            # ---------------- diff_pool 1 ----------------
            for g in range(NG):
                mps = psA.tile([K1, N1], F32, space="PSUM", tag="A", name="A")
                for vh in range(2):
                    nc.tensor.matmul(mps[:], lhsT=s1[:, g, vh, :], rhs=A1[:, g, vh, :],
                                     start=(vh == 0), stop=(vh == 1))
                msb = sb.tile([K1, N1], F16, tag="msb", name="msb")
                nc.vector.tensor_copy(msb[:], mps[:])
                n2t = sm.tile([128, 2, K1], F16, tag="n2t", name="n2t")
                for uh in range(2):
                    tps = psT.tile([128, K1], F16, space="PSUM", tag="T", name="T")
                    nc.tensor.transpose(tps[:], msb[:, uh * 128:(uh + 1) * 128],
                                        ident16[:K1, :K1])
                    nc.vector.tensor_scalar_mul(n2t[:, uh, :], tps[:],
                                                t1col[:, g, uh:uh + 1])
                oa_ps = psS.tile([K1, K1], F32, space="PSUM", tag="S", name="S")
                for uh in range(2):
                    nc.tensor.matmul(oa_ps[:], lhsT=n2t[:, uh, :], rhs=s1[:, g, uh, :],
                                     start=(uh == 0), stop=(uh == 1))
                nc.vector.tensor_copy(A2[:, g * N2:(g + 1) * N2], oa_ps[:])
                for j in range(3):
                    hp = psS.tile([H, K1], F32, space="PSUM", tag="S", name="S")
                    for uh in range(2):
                        nc.tensor.matmul(hp[:], lhsT=xr1[j][:, g, uh, 64:128],
                                         rhs=s1[:, g, uh, :],
                                         start=(uh == 0), stop=(uh == 1))
                    rk = psT.tile([H, K1], F32, space="PSUM", tag="T", name="T")
                    nc.tensor.matmul(rk[:], lhsT=brow1[j][:, 64:128],
                                     rhs=cs1[:, g * K1:(g + 1) * K1], start=True, stop=True)
                    dst = (h1a[64 * j:64 * (j + 1), g * N2:(g + 1) * N2] if j < 2
                           else h1b[:, g * N2:(g + 1) * N2])
                    tmp = sm.tile([H, K1], F32, tag="hfin", name="hfin")
                    nc.scalar.activation(tmp[:], hp[:], AF.Copy,
                                         scale=gam1[j][64:128, 0:1])
                    nc.vector.tensor_tensor(out=dst, in0=tmp[:], in1=rk[:], op=OP.add)
                for (src, c0, c1) in ((h1a, 0, 128), (h1b, 128, 192)):
                    cw = c1 - c0
                    tps = psT.tile([N2, 128], F16, space="PSUM", tag="T", name="T")
                    nc.tensor.transpose(tps[:, :cw], src[:cw, g * N2:(g + 1) * N2],
                                        ident16[:cw, :cw])
                    nc.scalar.copy(h1n[:, g, c0:c1], tps[:, :cw])
                d2r = psS.tile([1, N2], F32, space="PSUM", tag="S", name="S")
                nc.tensor.matmul(d2r[:], lhsT=ones_col[:N2], rhs=A2[:, g * N2:(g + 1) * N2],
                                 start=True, stop=True)
                t2r = sm.tile([1, N2], F16, tag="t2r", name="t2r")
                nc.vector.tensor_scalar_max(t2r[:], d2r[:], 1.0)
                d2c = psS.tile([N2, 1], F32, space="PSUM", tag="S", name="S")
                nc.tensor.matmul(d2c[:], lhsT=A2[:, g * N2:(g + 1) * N2],
                                 rhs=ones_col[:N2], start=True, stop=True)
                nc.vector.tensor_scalar_max(t2col[:, g:g + 1], d2c[:], 1.0)
                b2ps = psT.tile([N2, N2], F32, space="PSUM", tag="T", name="T")
                nc.tensor.matmul(b2ps[:], lhsT=ones_row[:, :N2], rhs=t2r[:],
                                 start=True, stop=True)
                rdb2 = sm.tile([N2, N2], F32, tag="rdb2", name="rdb2")
                nc.vector.reciprocal(rdb2[:], b2ps[:])
                nc.vector.tensor_tensor(out=A2[:, g * N2:(g + 1) * N2],
                                        in0=A2[:, g * N2:(g + 1) * N2],
                                        in1=rdb2[:], op=OP.mult)

            # ======================== PHASE 2 ========================
            p1.release()
            p23 = tc.alloc_tile_pool(name="p23", bufs=1)
            agg2a = p23.tile([128, NG * N2], F16, tag="agg2a", name="agg2a")
            agg2b = p23.tile([64, NG * N2], F16, tag="agg2b", name="agg2b")
            r2 = [p23.tile([ST2L[l], NG * N2], F16, tag=f"r2_{l}", name=f"r2_{l}") for l in range(3)]
            xr2 = [p23.tile([N2, NG, ST2L[l]], F16, tag=f"xr2_{l}", name=f"xr2_{l}") for l in range(3)]
            s2 = p23.tile([N2, NG * K2], F16, tag="s2", name="s2")
            cs2 = p23.tile([1, NG * K2], F16, tag="cs2", name="cs2")
            h2a = p23.tile([128, NG * N3], F16, tag="h2a", name="h2a")
            h2b = p23.tile([64, NG * N3], F16, tag="h2b", name="h2b")
            A3 = p23.tile([N3, NG * N3], F16, tag="A3", name="A3")
            t3col = p23.tile([N3, NG], F32, tag="t3col", name="t3col")
            h2n = p23.tile([N3, NG, 3 * H], F16, tag="h2n", name="h2n")
            agg3a = p23.tile([128, NG * N3], F16, tag="agg3a", name="agg3a")
            agg3b = p23.tile([64, NG * N3], F16, tag="agg3b", name="agg3b")
            r3 = [p23.tile([H, NG * N3], F16, tag=f"r3_{l}", name=f"r3_{l}") for l in range(3)]
            xr3 = [p23.tile([N3, NG, H], F16, tag=f"xr3_{l}", name=f"xr3_{l}") for l in range(3)]
            embA = p23.tile([128, NG], F16, tag="embA", name="embA")
            embB = p23.tile([64, NG], F16, tag="embB", name="embB")
            gam2, bet2, brow2 = [None] * 3, [None] * 3, [None] * 3
            for l in range(3):
                if l == 0:
                    for g in range(NG):
                        pa = psA.tile([128, N2], F32, space="PSUM", tag="A", name="A")
                        pb = psT.tile([64, N2], F32, space="PSUM", tag="T", name="T")
                        nc.tensor.matmul(pa[:], lhsT=h1n[:, g, 0:128],
                                         rhs=A2[:, g * N2:(g + 1) * N2],
                                         start=True, stop=True)
                        nc.tensor.matmul(pb[:], lhsT=h1n[:, g, 128:192],
                                         rhs=A2[:, g * N2:(g + 1) * N2],
                                         start=True, stop=True)
                        nc.vector.tensor_copy(agg2a[:, g * N2:(g + 1) * N2], pa[:])
                        nc.vector.tensor_copy(agg2b[:, g * N2:(g + 1) * N2], pb[:])
                else:
                    wr = scale_w(f"w2c{l + 1}r", gam2[l - 1])
                    wroot = scale_w(f"w2c{l + 1}root", gam2[l - 1])
                    bias = fold_bias(f"w2c{l + 1}r", f"w2c{l + 1}root",
                                     f"b2c{l + 1}", bet2[l - 1])
                    for g in range(NG):
                        pa = psA.tile([128, N2], F32, space="PSUM", tag="A", name="A")
                        nc.tensor.matmul(pa[:], lhsT=xr2[l - 1][:, g, :],
                                         rhs=A2[:, g * N2:(g + 1) * N2],
                                         start=True, stop=True)
                        nc.vector.tensor_copy(agg2a[:, g * N2:(g + 1) * N2], pa[:])
                C2 = ST2L[l]
                for gr in range(NGR2):
                    cols = slice(gr * G2W * N2, (gr + 1) * G2W * N2)
                    zps = psA.tile([C2, G2W * N2], F32, space="PSUM", tag="A", name="A")
                    if l == 0:
                        nc.tensor.matmul(zps[:], lhsT=wt["w2c1r_a"][:], rhs=agg2a[:, cols],
                                         start=True, stop=False)
                        nc.tensor.matmul(zps[:], lhsT=wt["w2c1r_b"][:], rhs=agg2b[:, cols],
                                         start=False, stop=False)
                        nc.tensor.matmul(zps[:], lhsT=wt["w2c1root_a"][:], rhs=h1a[:, cols],
                                         start=False, stop=False)
                        nc.tensor.matmul(zps[:], lhsT=wt["w2c1root_b"][:], rhs=h1b[:, cols],
                                         start=False, stop=True)
                        bias = wt["b2c1"]
                    else:
                        nc.tensor.matmul(zps[:], lhsT=wr[:], rhs=agg2a[:, cols],
                                         start=True, stop=False)
                        nc.tensor.matmul(zps[:], lhsT=wroot[:], rhs=r2[l - 1][:, cols],
                                         start=False, stop=True)
                    nc.scalar.activation(r2[l][:, cols], zps[:], AF.Relu,
                                         bias=bias[:], accum_out=sum2[l][:, gr:gr + 1])
                    sqs = sb.tile([128, 2 * N1], F16, tag="sqs", name="sqs")
                    nc.scalar.activation(sqs[:C2, :G2W * N2], r2[l][:, cols], AF.Square,
                                         accum_out=sq2[l][:, gr:gr + 1])
                for g in range(NG):
                    tps = psT.tile([N2, 128], F16, space="PSUM", tag="T", name="T")
                    nc.tensor.transpose(tps[:, :C2], r2[l][:, g * N2:(g + 1) * N2],
                                        ident16[:C2, :C2])
                    nc.scalar.copy(xr2[l][:, g, :], tps[:, :C2])
                gam2[l], bet2[l], brow2[l] = bn_sync(
                    f"p2l{l}", C2, sum2[l], sq2[l], NT2, f"bn2g{l + 1}", f"bn2b{l + 1}")

            # ---------------- lin2 -> s2 ----------------
            wlin2S = [scale_w(f"wlin2_{i}", gam2[i][0:PW2[i]]) for i in range(3)]
            lb2_ps = psS.tile([K2, 1], F32, space="PSUM", tag="S", name="S")
            for i in range(3):
                nc.tensor.matmul(lb2_ps[:], lhsT=wt[f"wlin2_{i}"][:],
                                 rhs=bet2[i][0:PW2[i]], start=(i == 0), stop=(i == 2))
            blin2 = sm.tile([K2, 1], F32, tag="blin2", name="blin2")
            nc.vector.tensor_tensor(out=blin2[:], in0=lb2_ps[:], in1=wt["blin2"][:], op=OP.add)
            for gr in range(NGR2):
                cols = slice(gr * G2W * N2, (gr + 1) * G2W * N2)
                sl_ps = psA.tile([K2, G2W * N2], F32, space="PSUM", tag="A", name="A")
                for i in range(3):
                    nc.tensor.matmul(sl_ps[:], lhsT=wlin2S[i][:],
                                     rhs=r2[i][0:PW2[i], cols], start=(i == 0), stop=(i == 2))
                slin2 = sb.tile([K2, G2W * N2], F16, tag="slin2", name="slin2")
                nc.scalar.activation(slin2[:], sl_ps[:], AF.Relu, bias=blin2[:])
                for gg in range(G2W):
                    g = gr * G2W + gg
                    tps = psT.tile([N2, K2], F16, space="PSUM", tag="T", name="T")
                    nc.tensor.transpose(tps[:], slin2[:, gg * N2:(gg + 1) * N2],
                                        ident16[:K2, :K2])
                    mx = sm.tile([N2, 1], F32, tag="s2mx", name="s2mx")
                    nc.vector.tensor_reduce(mx[:], tps[:], AX.X, OP.max)
                    nc.vector.tensor_scalar_mul(mx[:], mx[:], -1.0)
                    sme = sm.tile([N2, 1], F32, tag="s2me", name="s2me")
                    se = sm.tile([N2, K2], F32, tag="s2e", name="s2e")
                    nc.scalar.activation(se[:], tps[:], AF.Exp, bias=mx[:], accum_out=sme[:])
                    nc.vector.reciprocal(sme[:], sme[:])
                    nc.vector.tensor_scalar_mul(s2[:, g * K2:(g + 1) * K2], se[:], sme[:])
                    cs_ps = psS.tile([1, K2], F32, space="PSUM", tag="S", name="S")
                    nc.tensor.matmul(cs_ps[:], lhsT=ones_col[:N2],
                                     rhs=s2[:, g * K2:(g + 1) * K2], start=True, stop=True)
                    nc.vector.tensor_copy(cs2[:, g * K2:(g + 1) * K2], cs_ps[:])

            # ---------------- diff_pool 2 ----------------
            # gam2[2] embed slice starts at partition 16 (not 32-aligned);
            # DMA it to a base-0 tile for use as an ACT scale operand.
            gam2e2 = sm.tile([64, 1], F32, tag="gam2e2", name="gam2e2")
            nc.sync.dma_start(gam2e2[:], gam2[2][16:80, 0:1])
            for g in range(NG):
                gsl = slice(g * K2, (g + 1) * K2)
                nsl = slice(g * N2, (g + 1) * N2)
                n3sl = slice(g * N3, (g + 1) * N3)
                mps = psS.tile([K2, N2], F32, space="PSUM", tag="S", name="S")
                nc.tensor.matmul(mps[:], lhsT=s2[:, gsl], rhs=A2[:, nsl],
                                 start=True, stop=True)
                msb = sm.tile([K2, N2], F16, tag="m2sb", name="m2sb")
                nc.vector.tensor_copy(msb[:], mps[:])
                tps = psT.tile([N2, K2], F16, space="PSUM", tag="T", name="T")
                nc.tensor.transpose(tps[:], msb[:], ident16[:K2, :K2])
                n22 = sm.tile([N2, K2], F16, tag="n22", name="n22")
                nc.vector.tensor_scalar_mul(n22[:], tps[:], t2col[:, g:g + 1])
                oa_ps = psS.tile([K2, K2], F32, space="PSUM", tag="S", name="S")
                nc.tensor.matmul(oa_ps[:], lhsT=n22[:], rhs=s2[:, gsl], start=True, stop=True)
                nc.vector.tensor_copy(A3[:, n3sl], oa_ps[:])
                for j in range(3):
                    hp = psS.tile([H, K2], F32, space="PSUM", tag="S", name="S")
                    nc.tensor.matmul(hp[:], lhsT=xr2[j][:, g, PW2[j]:ST2L[j]],
                                     rhs=s2[:, gsl], start=True, stop=True)
                    rk = psT.tile([H, K2], F32, space="PSUM", tag="T", name="T")
                    nc.tensor.matmul(rk[:], lhsT=brow2[j][:, PW2[j]:ST2L[j]],
                                     rhs=cs2[:, gsl], start=True, stop=True)
                    dst = (h2a[64 * j:64 * (j + 1), n3sl] if j < 2 else h2b[:, n3sl])
                    tmp = sm.tile([H, K2], F32, tag="h2fin", name="h2fin")
                    scl = gam2e2[:, 0:1] if j == 2 else gam2[j][64:128, 0:1]
                    nc.scalar.activation(tmp[:], hp[:], AF.Copy, scale=scl)
                    nc.vector.tensor_tensor(out=dst, in0=tmp[:], in1=rk[:], op=OP.add)
                for (src, c0, c1) in ((h2a, 0, 128), (h2b, 128, 192)):
                    cw = c1 - c0
                    tps2 = psT.tile([N3, 128], F16, space="PSUM", tag="T", name="T")
                    nc.tensor.transpose(tps2[:, :cw], src[:cw, n3sl], ident16[:cw, :cw])
                    nc.scalar.copy(h2n[:, g, c0:c1], tps2[:, :cw])
                d3r = psS.tile([1, N3], F32, space="PSUM", tag="S", name="S")
                nc.tensor.matmul(d3r[:], lhsT=ones_col[:N3], rhs=A3[:, n3sl],
                                 start=True, stop=True)
                t3r = sm.tile([1, N3], F16, tag="t3r", name="t3r")
                nc.vector.tensor_scalar_max(t3r[:], d3r[:], 1.0)
                d3c = psS.tile([N3, 1], F32, space="PSUM", tag="S", name="S")
                nc.tensor.matmul(d3c[:], lhsT=A3[:, n3sl], rhs=ones_col[:N3],
                                 start=True, stop=True)
                nc.vector.tensor_scalar_max(t3col[:, g:g + 1], d3c[:], 1.0)
                b3ps = psT.tile([N3, N3], F32, space="PSUM", tag="T", name="T")
                nc.tensor.matmul(b3ps[:], lhsT=ones_row[:, :N3], rhs=t3r[:],
                                 start=True, stop=True)
                rdb3 = sm.tile([N3, N3], F32, tag="rdb3", name="rdb3")
                nc.vector.reciprocal(rdb3[:], b3ps[:])
                nc.vector.tensor_tensor(out=A3[:, n3sl], in0=A3[:, n3sl],
                                        in1=rdb3[:], op=OP.mult)

            # ======================== PHASE 3 ========================
            gam3, bet3 = [None] * 3, [None] * 3
            for l in range(3):
                if l == 0:
                    for g in range(NG):
                        n3sl = slice(g * N3, (g + 1) * N3)
                        pa = psA.tile([128, N3], F32, space="PSUM", tag="A", name="A")
                        pb = psT.tile([64, N3], F32, space="PSUM", tag="T", name="T")
                        nc.tensor.matmul(pa[:], lhsT=h2n[:, g, 0:128], rhs=A3[:, n3sl],
                                         start=True, stop=True)
                        nc.tensor.matmul(pb[:], lhsT=h2n[:, g, 128:192], rhs=A3[:, n3sl],
                                         start=True, stop=True)
                        nc.vector.tensor_copy(agg3a[:, n3sl], pa[:])
                        nc.vector.tensor_copy(agg3b[:, n3sl], pb[:])
                    zps = psA.tile([H, NG * N3], F32, space="PSUM", tag="A", name="A")
                    nc.tensor.matmul(zps[:], lhsT=wt["w3c1r_a"][:], rhs=agg3a[:],
                                     start=True, stop=False)
                    nc.tensor.matmul(zps[:], lhsT=wt["w3c1r_b"][:], rhs=agg3b[:],
                                     start=False, stop=False)
                    nc.tensor.matmul(zps[:], lhsT=wt["w3c1root_a"][:], rhs=h2a[:],
                                     start=False, stop=False)
                    nc.tensor.matmul(zps[:], lhsT=wt["w3c1root_b"][:], rhs=h2b[:],
                                     start=False, stop=True)
                    bias = wt["b3c1"]
                else:
                    wr = scale_w(f"w3c{l + 1}r", gam3[l - 1])
                    wroot = scale_w(f"w3c{l + 1}root", gam3[l - 1])
                    bias = fold_bias(f"w3c{l + 1}r", f"w3c{l + 1}root",
                                     f"b3c{l + 1}", bet3[l - 1])
                    for g in range(NG):
                        n3sl = slice(g * N3, (g + 1) * N3)
                        pa = psT.tile([H, N3], F32, space="PSUM", tag="T", name="T")
                        nc.tensor.matmul(pa[:], lhsT=xr3[l - 1][:, g, :], rhs=A3[:, n3sl],
                                         start=True, stop=True)
                        nc.vector.tensor_copy(agg3a[0:H, n3sl], pa[:])
                    zps = psA.tile([H, NG * N3], F32, space="PSUM", tag="A", name="A")
                    nc.tensor.matmul(zps[:], lhsT=wr[:], rhs=agg3a[0:H, :],
                                     start=True, stop=False)
                    nc.tensor.matmul(zps[:], lhsT=wroot[:], rhs=r3[l - 1][:],
                                     start=False, stop=True)
                nc.scalar.activation(r3[l][:], zps[:], AF.Relu, bias=bias[:],
                                     accum_out=sum3[l][:, 0:1])
                sqs = sb.tile([128, 2 * N1], F16, tag="sqs", name="sqs")
                nc.scalar.activation(sqs[0:H, :NG * N3], r3[l][:], AF.Square,
                                     accum_out=sq3[l][:, 0:1])
                for g in range(NG):
                    tps = psT.tile([N3, H], F16, space="PSUM", tag="T", name="T")
                    nc.tensor.transpose(tps[:], r3[l][:, g * N3:(g + 1) * N3],
                                        ident16[:H, :H])
                    nc.scalar.copy(xr3[l][:, g, :], tps[:])
                gam3[l], bet3[l], _ = bn_sync(
                    f"p3l{l}", H, sum3[l], sq3[l], NT3, f"bn3g{l + 1}", f"bn3b{l + 1}")

            # ---------------- emb -> lin1 -> lin2 -> log_softmax ----------------
            for j in range(3):
                t = sm.tile([H, NG], F32, tag="embred", name="embred")
                nc.vector.tensor_reduce(
                    t[:], r3[j][:].rearrange("c (g u) -> c g u", g=NG), AX.X, OP.add)
                g16 = sm.tile([H, 1], F32, tag="g16", name="g16")
                nc.vector.tensor_scalar_mul(g16[:], gam3[j][:], 1.0 / N3)
                dst = embA[64 * j:64 * (j + 1), :] if j < 2 else embB[:, :]
                nc.vector.scalar_tensor_tensor(
                    out=dst, in0=t[:], scalar=g16[:],
                    in1=bet3[j][:, 0:1].to_broadcast((H, NG)),
                    op0=OP.mult, op1=OP.add)
            l1ps = psS.tile([H, NG], F32, space="PSUM", tag="S", name="S")
            nc.tensor.matmul(l1ps[:], lhsT=wt["wlin1f_a"][:], rhs=embA[:],
                             start=True, stop=False)
            nc.tensor.matmul(l1ps[:], lhsT=wt["wlin1f_b"][:], rhs=embB[:],
                             start=False, stop=True)
            hf = sm.tile([H, NG], F16, tag="hf", name="hf")
            nc.scalar.activation(hf[:], l1ps[:], AF.Relu, bias=wt["blin1f"][:])
            l2ps = psS.tile([NCLS, NG], F32, space="PSUM", tag="S", name="S")
            nc.tensor.matmul(l2ps[:], lhsT=wt["wlin2f"][:], rhs=hf[:], start=True, stop=True)
            lg = sm.tile([NCLS, NG], F32, tag="lg", name="lg")
            nc.scalar.activation(lg[:], l2ps[:], AF.Identity, bias=wt["blin2f"][:])
            lgt_ps = psS.tile([NG, NCLS], F32, space="PSUM", tag="S", name="S")
            nc.tensor.transpose(lgt_ps[:], lg[:], ident32[:NCLS, :NCLS])
            lgt = sm.tile([NG, NCLS], F32, tag="lgt", name="lgt")
            nc.vector.tensor_copy(lgt[:], lgt_ps[:])
            mx = sm.tile([NG, 1], F32, tag="lmx", name="lmx")
            nc.vector.tensor_reduce(mx[:], lgt[:], AX.X, OP.max)
            nc.vector.tensor_scalar_mul(mx[:], mx[:], -1.0)
            esum = sm.tile([NG, 1], F32, tag="lesum", name="lesum")
            etmp = sm.tile([NG, NCLS], F32, tag="letmp", name="letmp")
            nc.scalar.activation(etmp[:], lgt[:], AF.Exp, bias=mx[:], accum_out=esum[:])
            lse = sm.tile([NG, 1], F32, tag="llse", name="llse")
            nc.scalar.activation(lse[:], esum[:], AF.Ln)
            outt = sm.tile([NG, NCLS], F32, tag="outt", name="outt")
            nc.vector.tensor_scalar(out=outt[:], in0=lgt[:], scalar1=mx[:],
                                    scalar2=lse[:], op0=OP.add, op1=OP.subtract)
            nc.sync.dma_start(out_d[:, :], outt[:])
            p23.release()

    nc.compile()
    return nc


_JIT = {}


def _run_cached(nc, in_maps):
    """run_bass_via_pjrt with the shard_map jit built once and reused."""
    import jax
    from jax.sharding import Mesh, PartitionSpec
    from jax.experimental.shard_map import shard_map
    from concourse import bass2jax
    import concourse.mybir as mb

    key = id(nc)
    if key not in _JIT:
        bass2jax.install_neuronx_cc_hook()
        in_names, out_names, out_avals, zero_shapes = [], [], [], []
        for alloc in nc.m.functions[0].allocations:
            if not isinstance(alloc, mb.MemoryLocationSet):
                continue
            name = alloc.memorylocations[0].name
            if alloc.kind == "ExternalInput":
                in_names.append(name)
            elif alloc.kind == "ExternalOutput":
                out_names.append(name)
                shape = tuple(alloc.tensor_shape)
                dtype = mb.dt.np(alloc.dtype)
                out_avals.append(jax.core.ShapedArray(shape, dtype))
                zero_shapes.append((shape, dtype))
        n_params = len(in_names)
        all_names = in_names + out_names

        def _body(*args):
            outs = bass2jax._bass_exec_p.bind(
                *args, out_avals=tuple(out_avals), in_names=tuple(all_names),
                out_names=tuple(out_names), lowering_input_output_aliases=(),
                sim_require_finite=True, sim_require_nnan=True, nc=nc)
            return tuple(outs)

        devices = jax.devices()[:N_CORES]
        mesh = Mesh(np.asarray(devices), ("core",))
        nio = n_params + len(out_names)
        fn = jax.jit(
            shard_map(_body, mesh=mesh,
                      in_specs=(PartitionSpec("core"),) * nio,
                      out_specs=(PartitionSpec("core"),) * len(out_names),
                      check_rep=False),
            donate_argnums=tuple(range(n_params, nio)), keep_unused=True)
        _JIT[key] = (fn, in_names, out_names, zero_shapes, out_avals)

    fn, in_names, out_names, zero_shapes, out_avals = _JIT[key]
    concat_in = [np.concatenate([np.asarray(m[n]) for m in in_maps], axis=0)
                 for n in in_names]
    concat_zeros = [np.zeros((N_CORES * s[0], *s[1:]), d) for s, d in zero_shapes]
    out_arrs = fn(*concat_in, *concat_zeros)
    return [
        {n: np.asarray(out_arrs[i]).reshape(N_CORES, *out_avals[i].shape)[c]
         for i, n in enumerate(out_names)}
        for c in range(N_CORES)
    ]


def kernel(x, batch, edge_index, params):
    in_maps, nidx, wshapes = _prep(x, batch, edge_index, params)
    key = (nidx, tuple(sorted(wshapes.items())))
    if key not in _CACHE:
        _CACHE[key] = _build(nidx, wshapes)
    nc = _CACHE[key]
    try:
        results = _run_cached(nc, in_maps)
    except Exception:
        res = run_bass_kernel_spmd(nc, in_maps, core_ids=list(range(N_CORES)))
        results = res.results
    return np.concatenate([r["out"] for r in results], axis=0)
